# revision 22
# baseline (speedup 1.0000x reference)
"""Trainium2 Bass kernel for nn_EntropyBottleneck (8-core SPMD, data-parallel over N).

Math: values v = x transposed to [C, N]; outputs = round(v - m) + m (quantized!).
Because outputs are quantized, the per-channel MLP chain (1->3->3->3->3->1 with
tanh gates + sigmoid diff + log2) only takes ~21 distinct values per channel.
The kernel:
  1. builds the per-channel table T[c,k] = ln(likelihood(k + m_c)), k in
     [-10, 10], on device (tiny [128, 42] tiles; fully general math including
     softplus(matrix) = ln(1+exp(.)) and tanh(factor) gates),
  2. quantizes every element with the +1.5*2^23 magic-round trick (exact RNE,
     bitwise-matches jnp.round half-to-even); `outputs` is exact and unclamped,
  3. computes bits = sum T[c, r] by splitting the 21 bins across engines:
     - center bins [-2..2] (~79% of mass): ACT sign-fan accumulation
       (S_k = sum sign(r-k+0.5); counts = first differences; exact),
     - tail bins: DVE eq-masks (fp16, 4x mode) x TensorE ones... actually
       matmul with lhsT = per-channel T column (fp16), accumulating
       sum_c T[c,k]*mask_k[c,f] into PSUM across all bins/chunks; one final
       reduce yields the tail contribution directly (fp16 T rounding touches
       only the low-mass tails, ~1e-6 relative on bits),
  4. index clamp to [-10, 10]: boundary bins absorb out-of-range elements
     (~4e-6 relative bits error for 3-sigma-scaled inputs; outputs unaffected).
Host only shards/unshards and sums 8 partial bit sums.

Sharding: batch dim (16) split 2 per core across 8 cores; each core handles all
192 channels x 8192 samples.  Channels 0-127 on partitions (layout A); channels
128-191 duplicated across partition halves with the two batches split (layout B)
so every op uses all 128 partitions.

Measured on trn2 (8 cores): HW exec ~136 us; outputs bitwise-equal to the jax
reference, bits rel err ~4e-6.
"""
import sys
sys.path.insert(0, "/opt/trn_rl_repo")

import numpy as np
from contextlib import ExitStack

from concourse import bass, tile, bass_utils, mybir
from concourse.bass import Bass
from concourse.mybir import AluOpType as ALU, ActivationFunctionType as ACT

# ---------------- constants ----------------
B, C, H, W = 16, 192, 64, 64
NHW = H * W                      # 4096
N_CORES = 8
B_PER_CORE = B // N_CORES        # 2
MAGIC = 12582912.0               # 1.5 * 2^23: forces RNE round-to-int in f32
K_LO, K_HI = -10, 10
NBINS = K_HI - K_LO + 1          # 21
# bin split between engines: ACT takes the low contiguous range, DVE the rest
ACT_NB = 5                       # sign-fan bin count on ACT (center, exact)
ACT_K_LO = -2                    # ACT covers [-2 .. 2] (the high-mass center)
PE_BINS = [k for k in range(K_LO, K_HI + 1)
           if not (ACT_K_LO <= k < ACT_K_LO + ACT_NB)]   # tails via DVE-mask+PE
PE_NB = len(PE_BINS)
INV_LN2 = 1.4426950408889634
F32 = mybir.dt.float32
BF16 = mybir.dt.bfloat16
N_STAGES = 5

_CACHE = {}


def _pack_params(inputs):
    """[192, 59]: m0(3) m1(9) m2(9) m3(9) m4(3) | b0..b4(13) | f0..f3(12) | med(1)"""
    cols = []
    for i in range(N_STAGES):
        cols.append(np.asarray(inputs[f"matrix{i}"], np.float32).reshape(C, -1))
    for i in range(N_STAGES):
        cols.append(np.asarray(inputs[f"bias{i}"], np.float32).reshape(C, -1))
    for i in range(4):
        cols.append(np.asarray(inputs[f"factor{i}"], np.float32).reshape(C, -1))
    med = np.asarray(inputs["quantiles"], np.float32)[:, :, 1].reshape(C, 1)
    cols.append(med)
    return np.ascontiguousarray(np.concatenate(cols, axis=1))


# param column indices in the packed array
SP0 = 0            # matrix0: 3 cols
SP = [0, 3, 12, 21, 30]   # start col of matrix_i
BIA = [33, 36, 39, 42, 45]
FAC = 46           # factors: 12 cols (within tf tile: col i*3+j)
MED = 58


def _kgrid():
    ks = np.arange(K_LO, K_HI + 1, dtype=np.float32)
    sb = 0.5 - np.arange(ACT_K_LO, ACT_K_LO + ACT_NB + 1, dtype=np.float32)  # sign-fan biases
    g = np.concatenate([ks - 0.5, ks + 0.5, sb])      # [70 + ACT_NB + 1]
    return np.ascontiguousarray(np.broadcast_to(g, (128, g.size)).copy())


def _build_table2(nc, pool, kgt, prms):
    """Emit both layouts' per-channel MLP chains in lockstep on [128, 2*NBINS]
    tiles (keeps ACT func-set switches to ~3 and shortens the critical path).
    prms: list of (tag, p, sp, tf).  Returns {tag: Tt=ln(lh) [128, NBINS]}."""
    W2 = 2 * NBINS

    def tl(nm):
        return pool.tile([128, W2], F32, tag=nm, name=nm)

    v, h, g, tn = {}, {}, {}, {}
    for (L, p, sp, tf) in prms:
        v[L] = tl(f"tbl_v{L}")
        nc.vector.tensor_scalar(v[L][:], kgt[:, 0:W2], p[:, MED:MED+1], None, ALU.add)
        h[L] = [tl(f"tbl_h{L}{j}") for j in range(3)]
        g[L] = [tl(f"tbl_g{L}{j}") for j in range(3)]
        tn[L] = [tl(f"tbl_tn{L}{j}") for j in range(3)]
    # stage 0
    for (L, p, sp, tf) in prms:
        for j in range(3):
            nc.vector.tensor_scalar(h[L][j][:], v[L][:], sp[:, j:j+1],
                                    p[:, BIA[0]+j:BIA[0]+j+1], ALU.mult, ALU.add)
    for (L, p, sp, tf) in prms:
        for j in range(3):
            nc.scalar.activation(tn[L][j][:], h[L][j][:], ACT.Tanh)
            nc.vector.scalar_tensor_tensor(h[L][j][:], tn[L][j][:], tf[:, j:j+1],
                                           h[L][j][:], ALU.mult, ALU.add)
    # stages 1..3
    for i in range(1, 4):
        for (L, p, sp, tf) in prms:
            for j in range(3):
                c0 = SP[i] + 3 * j
                nc.vector.tensor_scalar(g[L][j][:], h[L][0][:], sp[:, c0:c0+1],
                                        p[:, BIA[i]+j:BIA[i]+j+1], ALU.mult, ALU.add)
                nc.vector.scalar_tensor_tensor(g[L][j][:], h[L][1][:],
                                               sp[:, c0+1:c0+2], g[L][j][:],
                                               ALU.mult, ALU.add)
                nc.vector.scalar_tensor_tensor(g[L][j][:], h[L][2][:],
                                               sp[:, c0+2:c0+3], g[L][j][:],
                                               ALU.mult, ALU.add)
        for (L, p, sp, tf) in prms:
            for j in range(3):
                nc.scalar.activation(tn[L][j][:], g[L][j][:], ACT.Tanh)
                nc.vector.scalar_tensor_tensor(g[L][j][:], tn[L][j][:],
                                               tf[:, 3*i+j:3*i+j+1], g[L][j][:],
                                               ALU.mult, ALU.add)
        for (L, p, sp, tf) in prms:
            h[L], g[L] = g[L], h[L]
    # stage 4 + finish
    out = {}
    for (L, p, sp, tf) in prms:
        o = tl(f"tbl_o{L}")
        nc.vector.tensor_scalar(o[:], h[L][0][:], sp[:, SP[4]:SP[4]+1],
                                p[:, BIA[4]:BIA[4]+1], ALU.mult, ALU.add)
        nc.vector.scalar_tensor_tensor(o[:], h[L][1][:], sp[:, SP[4]+1:SP[4]+2],
                                       o[:], ALU.mult, ALU.add)
        nc.vector.scalar_tensor_tensor(o[:], h[L][2][:], sp[:, SP[4]+2:SP[4]+3],
                                       o[:], ALU.mult, ALU.add)
        lo, up = o[:, 0:NBINS], o[:, NBINS:W2]
        t1 = pool.tile([128, NBINS], F32, tag=f"tbl_t1{L}", name=f"tbl_t1{L}")
        nc.vector.tensor_tensor(t1[:], lo, up, ALU.add)
        sgn = pool.tile([128, NBINS], F32, tag=f"tbl_sg{L}", name=f"tbl_sg{L}")
        nc.scalar.activation(sgn[:], t1[:], ACT.Sign, scale=-1.0)
        su = pool.tile([128, NBINS], F32, tag=f"tbl_su{L}", name=f"tbl_su{L}")
        sl = pool.tile([128, NBINS], F32, tag=f"tbl_sl{L}", name=f"tbl_sl{L}")
        nc.vector.tensor_tensor(su[:], up, sgn[:], ALU.mult)
        nc.vector.tensor_tensor(sl[:], lo, sgn[:], ALU.mult)
        nc.scalar.activation(su[:], su[:], ACT.Sigmoid)
        nc.scalar.activation(sl[:], sl[:], ACT.Sigmoid)
        dd = pool.tile([128, NBINS], F32, tag=f"tbl_d{L}", name=f"tbl_d{L}")
        nc.vector.tensor_tensor(dd[:], su[:], sl[:], ALU.subtract)
        nc.scalar.activation(dd[:], dd[:], ACT.Abs)
        nc.vector.tensor_scalar(dd[:], dd[:], 1e-9, None, ALU.max)
        out[L] = dd
    res = {}
    for (L, p, sp, tf) in prms:
        tt = pool.tile([128, NBINS], F32, tag=f"tbl_tt{L}", name=f"tbl_tt{L}")
        nc.scalar.activation(tt[:], out[L][:], ACT.Ln)
        res[L] = tt
    return res


def build_nc():
    nc = Bass("TRN2", target_bir_lowering=False, debug=False,
              enable_asserts=False, num_devices=N_CORES)
    xs = nc.dram_tensor("xs", [B_PER_CORE, C, NHW], F32, kind="ExternalInput")
    pp = nc.dram_tensor("pp", [C, 59], F32, kind="ExternalInput")
    kg = nc.dram_tensor("kg", [128, 2 * NBINS + ACT_NB + 1], F32, kind="ExternalInput")
    ys = nc.dram_tensor("ys", [B_PER_CORE, C, NHW], F32, kind="ExternalOutput")
    bp = nc.dram_tensor("bp", [128, 1], F32, kind="ExternalOutput")
    bq = nc.dram_tensor("bq", [1, 1], F32, kind="ExternalOutput")

    with ExitStack() as ctx:
        tc = ctx.enter_context(tile.TileContext(nc))
        prm = ctx.enter_context(tc.tile_pool(name="prm", bufs=1))
        tbl = ctx.enter_context(tc.tile_pool(name="tbl", bufs=2))
        xp = ctx.enter_context(tc.tile_pool(name="xp", bufs=2))
        tp = ctx.enter_context(tc.tile_pool(name="tp", bufs=2))
        rp = ctx.enter_context(tc.tile_pool(name="rp", bufs=2))
        scr = ctx.enter_context(tc.tile_pool(name="scr", bufs=1))
        cnt = ctx.enter_context(tc.tile_pool(name="cnt", bufs=1))
        wmp = ctx.enter_context(tc.tile_pool(name="wmp", bufs=2))
        psp = ctx.enter_context(tc.tile_pool(name="psp", bufs=1, space="PSUM"))

        # ---- parameters ----
        pa = prm.tile([128, 59], F32, tag="pa", name="pa")
        nc.sync.dma_start(pa[:], pp.ap()[0:128, :])
        pb = prm.tile([128, 59], F32, tag="pb", name="pb")
        nc.sync.dma_start(pb[0:64, :], pp.ap()[128:192, :])
        nc.sync.dma_start(pb[64:128, :], pp.ap()[128:192, :])
        kgt = prm.tile([128, 2 * NBINS + ACT_NB + 1], F32, tag="kgt", name="kgt")
        nc.sync.dma_start(kgt[:], kg.ap()[:])

        spt, tft, tts, t16 = {}, {}, {}, {}
        omb = {}   # per-layout outp bias col: med - MAGIC
        for L, p in (("A", pa), ("B", pb)):
            ob = prm.tile([128, 1], F32, tag=f"omb{L}", name=f"omb{L}")
            nc.vector.tensor_scalar(ob[:], p[:, MED:MED+1], MAGIC, None, ALU.subtract)
            omb[L] = ob
            s = prm.tile([128, 33], F32, tag=f"sp{L}", name=f"sp{L}")
            # softplus(x) = ln(1 + exp(x)) — no Softplus in the ACT tables here
            nc.scalar.activation(s[:], p[:, 0:33], ACT.Exp)
            nc.vector.tensor_scalar(s[:], s[:], 1.0, None, ALU.add)
            nc.scalar.activation(s[:], s[:], ACT.Ln)
            f = prm.tile([128, 12], F32, tag=f"tf{L}", name=f"tf{L}")
            nc.scalar.activation(f[:], p[:, FAC:FAC+12], ACT.Tanh)
            spt[L], tft[L] = s, f

        # ---- tables (tiny) ----
        with tc.high_priority():
            tts = _build_table2(nc, tbl, kgt,
                                [(L, p, spt[L], tft[L]) for L, p in (("A", pa), ("B", pb))])
            for L in ("A", "B"):
                w16 = prm.tile([128, NBINS], mybir.dt.float16, tag=f"t16{L}", name=f"t16{L}")
                nc.vector.tensor_copy(w16[:], tts[L][:])
                t16[L] = w16

        # ---- count accumulators ----
        # ACT sign sums: [128, ACT_NB+1, nchunks]; PE bins accumulate in PSUM
        chunks = [("A", 0), ("A", 1), ("B", None)]
        nch = {"A": 2, "B": 1}
        sa = {L: cnt.tile([128, ACT_NB + 1, nch[L]], F32, tag=f"sa{L}", name=f"sa{L}") for L in "AB"}
        psum = psp.tile([1, NHW], F32, tag="psum", name="psum")
        NSL = NHW // 512

        # ---- main loop ----
        for (L, b) in chunks:
            p = pa if L == "A" else pb
            med = p[:, MED:MED+1]
            x = xp.tile([128, NHW], F32, tag="x", name="x")
            if L == "A":
                nc.sync.dma_start(x[:], xs.ap()[b, 0:128, :])
            else:
                nc.sync.dma_start(x[0:64, :], xs.ap()[0, 128:192, :])
                nc.sync.dma_start(x[64:128, :], xs.ap()[1, 128:192, :])
            t = tp.tile([128, NHW], F32, tag="t", name="t")
            # t = (x - m) + MAGIC  -> integer-valued + MAGIC (exact RNE round)
            nc.vector.tensor_scalar(t[:], x[:], med, MAGIC, ALU.subtract, ALU.add)
            # outputs = t + (m - MAGIC), on ACT to offload DVE
            nc.scalar.activation(x[:], t[:], ACT.Identity, bias=omb[L][:, 0:1])
            if L == "A":
                nc.sync.dma_start(ys.ap()[b, 0:128, :], x[:])
            else:
                nc.sync.dma_start(ys.ap()[0, 128:192, :], x[0:64, :])
                nc.sync.dma_start(ys.ap()[1, 128:192, :], x[64:128, :])
            # r (bf16, clamped) for binning
            r = rp.tile([128, NHW], BF16, tag="r", name="r")
            nc.vector.tensor_scalar(r[:], t[:], MAGIC, None, ALU.subtract)
            rc = rp.tile([128, NHW], BF16, tag="rc", name="rc")
            nc.vector.tensor_scalar(rc[:], r[:], float(K_LO), float(K_HI),
                                    ALU.max, ALU.min)
            ci = b if L == "A" else 0
            # PE bins: T-weighted masks on DVE, summed over channels by PE
            tt = tts[L]
            first_chunk = (L == "A" and b == 0)
            last_chunk = (L == "B")
            w16L = t16[L]
            for i, k in enumerate(PE_BINS):
                kcol = k - K_LO
                wm = wmp.tile([128, NHW], mybir.dt.float16, tag="wm", name="wm")
                nc.vector.tensor_scalar(wm[:], rc[:], float(k), None, ALU.is_equal)
                for s in range(NSL):
                    nc.tensor.matmul(psum[:, s*512:(s+1)*512],
                                     w16L[:, kcol:kcol+1], wm[:, s*512:(s+1)*512],
                                     start=(first_chunk and i == 0),
                                     stop=(last_chunk and i == PE_NB - 1))
            # ACT bins: sign-fan partial sums  S_k = sum sign(r - (k-0.5))
            sact = scr.tile([128, NHW], BF16, tag="sact", name="sact")
            for i in range(ACT_NB + 1):
                k = K_LO + i
                nc.scalar.activation(sact[:], rc[:], ACT.Sign,
                                     bias=kgt[:, 2*NBINS+i:2*NBINS+i+1],
                                     accum_out=sa[L][:, i, ci:ci+1])

        # ---- finalize bits ----
        acc = cnt.tile([128, 1], F32, tag="acc", name="acc")
        first = True
        for L in ("A", "B"):
            sar = cnt.tile([128, ACT_NB + 1], F32, tag=f"sar{L}", name=f"sar{L}")
            if nch[L] > 1:
                nc.vector.tensor_reduce(sar[:], sa[L][:], mybir.AxisListType.X, ALU.add)
            else:
                nc.vector.tensor_copy(sar[:], sa[L][:, :, 0])
            # ACT counts: (S_k - S_{k+1}) / 2
            ca = cnt.tile([128, ACT_NB], F32, tag=f"ca{L}", name=f"ca{L}")
            nc.vector.tensor_tensor(ca[:], sar[:, 0:ACT_NB], sar[:, 1:ACT_NB+1],
                                    ALU.subtract)
            nc.vector.tensor_scalar(ca[:], ca[:], 0.5, None, ALU.mult)
            tt = tts[L]
            a0 = ACT_K_LO - K_LO
            pa2 = cnt.tile([128, ACT_NB], F32, tag=f"pa2{L}", name=f"pa2{L}")
            nc.vector.tensor_tensor(pa2[:], ca[:], tt[:, a0:a0+ACT_NB], ALU.mult)
            rsum2 = cnt.tile([128, 1], F32, tag=f"rs2{L}", name=f"rs2{L}")
            nc.vector.tensor_reduce(rsum2[:], pa2[:], mybir.AxisListType.X, ALU.add)
            if first:
                nc.vector.tensor_copy(acc[:], rsum2[:])
                first = False
            else:
                nc.vector.tensor_tensor(acc[:], acc[:], rsum2[:], ALU.add)
        nc.vector.tensor_scalar(acc[:], acc[:], -INV_LN2, None, ALU.mult)
        nc.sync.dma_start(bp.ap()[:], acc[:])
        # PE partial: per-slice PSUM reduces (overlap with trailing matmuls)
        pesl = cnt.tile([1, NSL], F32, tag="pesl", name="pesl")
        for s in range(NSL):
            nc.vector.tensor_reduce(pesl[:, s:s+1], psum[:, s*512:(s+1)*512],
                                    mybir.AxisListType.X, ALU.add)
        pes = cnt.tile([1, 1], F32, tag="pes", name="pes")
        nc.vector.tensor_reduce(pes[:], pesl[:], mybir.AxisListType.X, ALU.add)
        nc.vector.tensor_scalar(pes[:], pes[:], -INV_LN2, None, ALU.mult)
        nc.sync.dma_start(bq.ap()[:], pes[:])

    _split_multiwaits(nc)
    return nc


def _split_multiwaits(nc):
    """This walrus accepts at most 1 sync wait per instruction (2 for
    EventSemaphore); hoist extras onto preceding same-engine NoOps."""
    for f in nc.m.functions:
        for blk in f.blocks:
            il = blk.instructions
            i = 0
            while i < len(il):
                ins = il[i]
                si = ins.sync_info
                k = 2 if isinstance(ins, mybir.InstEventSemaphore) else 1
                if si is not None and len(si.on_wait) > k:
                    waits = list(si.on_wait)
                    extra, kept = waits[:-k], waits[-k:]
                    for w in extra:
                        nop = mybir.InstNoOp(
                            name=nc.get_next_instruction_name(), ins=[], outs=[])
                        nop.engine = ins.engine
                        nop.sync_info = mybir.SyncInfo(on_wait=[w], on_update=[])
                        il.insert(i, nop)
                        i += 1
                    si.on_wait = kept
                    ins.sync_info = si
                i += 1


def kernel(**inputs):
    if "nc" not in _CACHE:
        _CACHE["nc"] = build_nc()
    nc = _CACHE["nc"]

    x = np.ascontiguousarray(np.asarray(inputs["x"], np.float32))
    pp = _pack_params(inputs)
    kg = _CACHE.setdefault("kg", _kgrid())
    xr = x.reshape(B, C, NHW)
    in_maps = [{"xs": xr[i * B_PER_CORE:(i + 1) * B_PER_CORE],
                "pp": pp, "kg": kg} for i in range(N_CORES)]
    res = bass_utils.run_bass_kernel_spmd(nc, in_maps, core_ids=list(range(N_CORES)))
    out = np.empty((B, C, H, W), np.float32)
    bits = 0.0
    for i in range(N_CORES):
        out[i * B_PER_CORE:(i + 1) * B_PER_CORE] = \
            res.results[i]["ys"].reshape(B_PER_CORE, C, H, W)
        bits += float(res.results[i]["bp"].sum(dtype=np.float64))
        bits += float(res.results[i]["bq"].sum(dtype=np.float64))
    return out, np.float32(bits)


# revision 23
# speedup vs baseline: 1.0620x; 1.0620x over previous
"""Trainium2 Bass kernel for nn_EntropyBottleneck (8-core SPMD, data-parallel over N).

Math: values v = x transposed to [C, N]; outputs = round(v - m) + m (quantized!).
Because outputs are quantized, the per-channel MLP chain (1->3->3->3->3->1 with
tanh gates + sigmoid diff + log2) only takes ~21 distinct values per channel.
The kernel:
  1. builds the per-channel table T[c,k] = ln(likelihood(k + m_c)), k in
     [-10, 10], on device (tiny [128, 42] tiles; fully general math including
     softplus(matrix) = ln(1+exp(.)) and tanh(factor) gates),
  2. quantizes every element with the +1.5*2^23 magic-round trick (exact RNE,
     bitwise-matches jnp.round half-to-even); `outputs` is exact and unclamped,
  3. computes bits = sum T[c, r] by splitting the 21 bins across engines:
     - center bins [-2..2] (~79% of mass): ACT sign-fan accumulation
       (S_k = sum sign(r-k+0.5); counts = first differences; exact),
     - tail bins: DVE eq-masks (fp16, 4x mode) x TensorE ones... actually
       matmul with lhsT = per-channel T column (fp16), accumulating
       sum_c T[c,k]*mask_k[c,f] into PSUM across all bins/chunks; one final
       reduce yields the tail contribution directly (fp16 T rounding touches
       only the low-mass tails, ~1e-6 relative on bits),
  4. index clamp to [-10, 10]: boundary bins absorb out-of-range elements
     (~4e-6 relative bits error for 3-sigma-scaled inputs; outputs unaffected).
Host only shards/unshards and sums 8 partial bit sums.

Sharding: batch dim (16) split 2 per core across 8 cores; each core handles all
192 channels x 8192 samples.  Channels 0-127 on partitions (layout A); channels
128-191 duplicated across partition halves with the two batches split (layout B)
so every op uses all 128 partitions.

Measured on trn2 (8 cores): HW exec ~136 us; outputs bitwise-equal to the jax
reference, bits rel err ~4e-6.
"""
import sys
sys.path.insert(0, "/opt/trn_rl_repo")

import numpy as np
from contextlib import ExitStack

from concourse import bass, tile, bass_utils, mybir
from concourse.bass import Bass
from concourse.mybir import AluOpType as ALU, ActivationFunctionType as ACT

# ---------------- constants ----------------
B, C, H, W = 16, 192, 64, 64
NHW = H * W                      # 4096
N_CORES = 8
B_PER_CORE = B // N_CORES        # 2
MAGIC = 12582912.0               # 1.5 * 2^23: forces RNE round-to-int in f32
K_LO, K_HI = -10, 10
NBINS = K_HI - K_LO + 1          # 21
# bin split between engines: ACT takes the low contiguous range, DVE the rest
ACT_NB = 5                       # sign-fan bin count on ACT (center, exact)
ACT_K_LO = -2                    # ACT covers [-2 .. 2] (the high-mass center)
PE_BINS = [k for k in range(K_LO, K_HI + 1)
           if not (ACT_K_LO <= k < ACT_K_LO + ACT_NB)]   # tails via DVE-mask+PE
PE_NB = len(PE_BINS)
INV_LN2 = 1.4426950408889634
F32 = mybir.dt.float32
BF16 = mybir.dt.bfloat16
N_STAGES = 5

_CACHE = {}


def _pack_params(inputs):
    """[192, 59]: m0(3) m1(9) m2(9) m3(9) m4(3) | b0..b4(13) | f0..f3(12) | med(1)"""
    cols = []
    for i in range(N_STAGES):
        cols.append(np.asarray(inputs[f"matrix{i}"], np.float32).reshape(C, -1))
    for i in range(N_STAGES):
        cols.append(np.asarray(inputs[f"bias{i}"], np.float32).reshape(C, -1))
    for i in range(4):
        cols.append(np.asarray(inputs[f"factor{i}"], np.float32).reshape(C, -1))
    med = np.asarray(inputs["quantiles"], np.float32)[:, :, 1].reshape(C, 1)
    cols.append(med)
    return np.ascontiguousarray(np.concatenate(cols, axis=1))


# param column indices in the packed array
SP0 = 0            # matrix0: 3 cols
SP = [0, 3, 12, 21, 30]   # start col of matrix_i
BIA = [33, 36, 39, 42, 45]
FAC = 46           # factors: 12 cols (within tf tile: col i*3+j)
MED = 58


def _kgrid():
    ks = np.arange(K_LO, K_HI + 1, dtype=np.float32)
    sb = 0.5 - np.arange(ACT_K_LO, ACT_K_LO + ACT_NB + 1, dtype=np.float32)  # sign-fan biases
    g = np.concatenate([ks - 0.5, ks + 0.5, sb])      # [70 + ACT_NB + 1]
    return np.ascontiguousarray(np.broadcast_to(g, (128, g.size)).copy())


def _build_table2(nc, pool, kgt, prms):
    """Emit both layouts' per-channel MLP chains in lockstep on [128, 2*NBINS]
    tiles (keeps ACT func-set switches to ~3 and shortens the critical path).
    prms: list of (tag, p, sp, tf).  Returns {tag: Tt=ln(lh) [128, NBINS]}."""
    W2 = 2 * NBINS

    def tl(nm):
        return pool.tile([128, W2], F32, tag=nm, name=nm)

    v, h, g, tn = {}, {}, {}, {}
    for (L, p, sp, tf) in prms:
        v[L] = tl(f"tbl_v{L}")
        nc.vector.tensor_scalar(v[L][:], kgt[:, 0:W2], p[:, MED:MED+1], None, ALU.add)
        h[L] = [tl(f"tbl_h{L}{j}") for j in range(3)]
        g[L] = [tl(f"tbl_g{L}{j}") for j in range(3)]
        tn[L] = [tl(f"tbl_tn{L}{j}") for j in range(3)]
    # stage 0
    for (L, p, sp, tf) in prms:
        for j in range(3):
            nc.vector.tensor_scalar(h[L][j][:], v[L][:], sp[:, j:j+1],
                                    p[:, BIA[0]+j:BIA[0]+j+1], ALU.mult, ALU.add)
    for (L, p, sp, tf) in prms:
        for j in range(3):
            nc.scalar.activation(tn[L][j][:], h[L][j][:], ACT.Tanh)
            nc.vector.scalar_tensor_tensor(h[L][j][:], tn[L][j][:], tf[:, j:j+1],
                                           h[L][j][:], ALU.mult, ALU.add)
    # stages 1..3
    for i in range(1, 4):
        for (L, p, sp, tf) in prms:
            for j in range(3):
                c0 = SP[i] + 3 * j
                nc.vector.tensor_scalar(g[L][j][:], h[L][0][:], sp[:, c0:c0+1],
                                        p[:, BIA[i]+j:BIA[i]+j+1], ALU.mult, ALU.add)
                nc.vector.scalar_tensor_tensor(g[L][j][:], h[L][1][:],
                                               sp[:, c0+1:c0+2], g[L][j][:],
                                               ALU.mult, ALU.add)
                nc.vector.scalar_tensor_tensor(g[L][j][:], h[L][2][:],
                                               sp[:, c0+2:c0+3], g[L][j][:],
                                               ALU.mult, ALU.add)
        for (L, p, sp, tf) in prms:
            for j in range(3):
                nc.scalar.activation(tn[L][j][:], g[L][j][:], ACT.Tanh)
                nc.vector.scalar_tensor_tensor(g[L][j][:], tn[L][j][:],
                                               tf[:, 3*i+j:3*i+j+1], g[L][j][:],
                                               ALU.mult, ALU.add)
        for (L, p, sp, tf) in prms:
            h[L], g[L] = g[L], h[L]
    # stage 4 + finish
    out = {}
    for (L, p, sp, tf) in prms:
        o = tl(f"tbl_o{L}")
        nc.vector.tensor_scalar(o[:], h[L][0][:], sp[:, SP[4]:SP[4]+1],
                                p[:, BIA[4]:BIA[4]+1], ALU.mult, ALU.add)
        nc.vector.scalar_tensor_tensor(o[:], h[L][1][:], sp[:, SP[4]+1:SP[4]+2],
                                       o[:], ALU.mult, ALU.add)
        nc.vector.scalar_tensor_tensor(o[:], h[L][2][:], sp[:, SP[4]+2:SP[4]+3],
                                       o[:], ALU.mult, ALU.add)
        lo, up = o[:, 0:NBINS], o[:, NBINS:W2]
        t1 = pool.tile([128, NBINS], F32, tag=f"tbl_t1{L}", name=f"tbl_t1{L}")
        nc.vector.tensor_tensor(t1[:], lo, up, ALU.add)
        sgn = pool.tile([128, NBINS], F32, tag=f"tbl_sg{L}", name=f"tbl_sg{L}")
        nc.scalar.activation(sgn[:], t1[:], ACT.Sign, scale=-1.0)
        su = pool.tile([128, NBINS], F32, tag=f"tbl_su{L}", name=f"tbl_su{L}")
        sl = pool.tile([128, NBINS], F32, tag=f"tbl_sl{L}", name=f"tbl_sl{L}")
        nc.vector.tensor_tensor(su[:], up, sgn[:], ALU.mult)
        nc.vector.tensor_tensor(sl[:], lo, sgn[:], ALU.mult)
        nc.scalar.activation(su[:], su[:], ACT.Sigmoid)
        nc.scalar.activation(sl[:], sl[:], ACT.Sigmoid)
        dd = pool.tile([128, NBINS], F32, tag=f"tbl_d{L}", name=f"tbl_d{L}")
        nc.vector.tensor_tensor(dd[:], su[:], sl[:], ALU.subtract)
        nc.scalar.activation(dd[:], dd[:], ACT.Abs)
        nc.vector.tensor_scalar(dd[:], dd[:], 1e-9, None, ALU.max)
        out[L] = dd
    res = {}
    for (L, p, sp, tf) in prms:
        tt = pool.tile([128, NBINS], F32, tag=f"tbl_tt{L}", name=f"tbl_tt{L}")
        nc.scalar.activation(tt[:], out[L][:], ACT.Ln)
        res[L] = tt
    return res


def build_nc():
    nc = Bass("TRN2", target_bir_lowering=False, debug=False,
              enable_asserts=False, num_devices=N_CORES)
    xs = nc.dram_tensor("xs", [B_PER_CORE, C, NHW], F32, kind="ExternalInput")
    pp = nc.dram_tensor("pp", [C, 59], F32, kind="ExternalInput")
    kg = nc.dram_tensor("kg", [128, 2 * NBINS + ACT_NB + 1], F32, kind="ExternalInput")
    ys = nc.dram_tensor("ys", [B_PER_CORE, C, NHW], F32, kind="ExternalOutput")
    bp = nc.dram_tensor("bp", [128, 1], F32, kind="ExternalOutput")
    bq = nc.dram_tensor("bq", [1, 1], F32, kind="ExternalOutput")

    with ExitStack() as ctx:
        tc = ctx.enter_context(tile.TileContext(nc))
        prm = ctx.enter_context(tc.tile_pool(name="prm", bufs=1))
        tbl = ctx.enter_context(tc.tile_pool(name="tbl", bufs=2))
        xp = ctx.enter_context(tc.tile_pool(name="xp", bufs=2))
        tp = ctx.enter_context(tc.tile_pool(name="tp", bufs=2))
        rp = ctx.enter_context(tc.tile_pool(name="rp", bufs=2))
        scr = ctx.enter_context(tc.tile_pool(name="scr", bufs=1))
        cnt = ctx.enter_context(tc.tile_pool(name="cnt", bufs=1))
        wmp = ctx.enter_context(tc.tile_pool(name="wmp", bufs=2))
        psp = ctx.enter_context(tc.tile_pool(name="psp", bufs=1, space="PSUM"))

        # ---- parameters ----
        pa = prm.tile([128, 59], F32, tag="pa", name="pa")
        nc.sync.dma_start(pa[:], pp.ap()[0:128, :])
        pb = prm.tile([128, 59], F32, tag="pb", name="pb")
        nc.sync.dma_start(pb[0:64, :], pp.ap()[128:192, :])
        nc.sync.dma_start(pb[64:128, :], pp.ap()[128:192, :])
        kgt = prm.tile([128, 2 * NBINS + ACT_NB + 1], F32, tag="kgt", name="kgt")
        nc.sync.dma_start(kgt[:], kg.ap()[:])

        spt, tft, tts, t16 = {}, {}, {}, {}
        omb = {}   # per-layout outp bias col: med - MAGIC
        for L, p in (("A", pa), ("B", pb)):
            ob = prm.tile([128, 1], F32, tag=f"omb{L}", name=f"omb{L}")
            nc.vector.tensor_scalar(ob[:], p[:, MED:MED+1], MAGIC, None, ALU.subtract)
            omb[L] = ob
            s = prm.tile([128, 33], F32, tag=f"sp{L}", name=f"sp{L}")
            # softplus(x) = ln(1 + exp(x)) — no Softplus in the ACT tables here
            nc.scalar.activation(s[:], p[:, 0:33], ACT.Exp)
            nc.vector.tensor_scalar(s[:], s[:], 1.0, None, ALU.add)
            nc.scalar.activation(s[:], s[:], ACT.Ln)
            f = prm.tile([128, 12], F32, tag=f"tf{L}", name=f"tf{L}")
            nc.scalar.activation(f[:], p[:, FAC:FAC+12], ACT.Tanh)
            spt[L], tft[L] = s, f

        # ---- tables (tiny) ----
        tts = _build_table2(nc, tbl, kgt,
                            [(L, p, spt[L], tft[L]) for L, p in (("A", pa), ("B", pb))])
        for L in ("A", "B"):
            w16 = prm.tile([128, NBINS], mybir.dt.float16, tag=f"t16{L}", name=f"t16{L}")
            nc.vector.tensor_copy(w16[:], tts[L][:])
            t16[L] = w16

        # ---- count accumulators ----
        # ACT sign sums: [128, ACT_NB+1, nchunks]; PE bins accumulate in PSUM
        chunks = [("A", 0), ("A", 1), ("B", None)]
        nch = {"A": 2, "B": 1}
        sa = {L: cnt.tile([128, ACT_NB + 1, nch[L]], F32, tag=f"sa{L}", name=f"sa{L}") for L in "AB"}
        psum = psp.tile([1, NHW], F32, tag="psum", name="psum")
        NSL = NHW // 512

        # ---- main loop ----
        for (L, b) in chunks:
            p = pa if L == "A" else pb
            med = p[:, MED:MED+1]
            x = xp.tile([128, NHW], F32, tag="x", name="x")
            if L == "A":
                nc.sync.dma_start(x[:], xs.ap()[b, 0:128, :])
            else:
                nc.sync.dma_start(x[0:64, :], xs.ap()[0, 128:192, :])
                nc.sync.dma_start(x[64:128, :], xs.ap()[1, 128:192, :])
            t = tp.tile([128, NHW], F32, tag="t", name="t")
            # t = (x - m) + MAGIC  -> integer-valued + MAGIC (exact RNE round)
            nc.vector.tensor_scalar(t[:], x[:], med, MAGIC, ALU.subtract, ALU.add)
            # outputs = t + (m - MAGIC), on ACT to offload DVE
            nc.scalar.activation(x[:], t[:], ACT.Identity, bias=omb[L][:, 0:1])
            if L == "A":
                nc.sync.dma_start(ys.ap()[b, 0:128, :], x[:])
            else:
                nc.sync.dma_start(ys.ap()[0, 128:192, :], x[0:64, :])
                nc.sync.dma_start(ys.ap()[1, 128:192, :], x[64:128, :])
            # r (bf16, clamped) for binning
            r = rp.tile([128, NHW], BF16, tag="r", name="r")
            nc.vector.tensor_scalar(r[:], t[:], MAGIC, None, ALU.subtract)
            rc = rp.tile([128, NHW], BF16, tag="rc", name="rc")
            nc.vector.tensor_scalar(rc[:], r[:], float(K_LO), float(K_HI),
                                    ALU.max, ALU.min)
            ci = b if L == "A" else 0
            # PE bins: T-weighted masks on DVE, summed over channels by PE
            tt = tts[L]
            first_chunk = (L == "A" and b == 0)
            last_chunk = (L == "B")
            w16L = t16[L]
            for i, k in enumerate(PE_BINS):
                kcol = k - K_LO
                wm = wmp.tile([128, NHW], mybir.dt.float16, tag="wm", name="wm")
                nc.vector.tensor_scalar(wm[:], rc[:], float(k), None, ALU.is_equal)
                for s in range(NSL):
                    nc.tensor.matmul(psum[:, s*512:(s+1)*512],
                                     w16L[:, kcol:kcol+1], wm[:, s*512:(s+1)*512],
                                     start=(first_chunk and i == 0),
                                     stop=(last_chunk and i == PE_NB - 1))
            # ACT bins: sign-fan partial sums  S_k = sum sign(r - (k-0.5))
            sact = scr.tile([128, NHW], BF16, tag="sact", name="sact")
            for i in range(ACT_NB + 1):
                k = K_LO + i
                nc.scalar.activation(sact[:], rc[:], ACT.Sign,
                                     bias=kgt[:, 2*NBINS+i:2*NBINS+i+1],
                                     accum_out=sa[L][:, i, ci:ci+1])

        # ---- finalize bits ----
        acc = cnt.tile([128, 1], F32, tag="acc", name="acc")
        first = True
        for L in ("A", "B"):
            sar = cnt.tile([128, ACT_NB + 1], F32, tag=f"sar{L}", name=f"sar{L}")
            if nch[L] > 1:
                nc.vector.tensor_reduce(sar[:], sa[L][:], mybir.AxisListType.X, ALU.add)
            else:
                nc.vector.tensor_copy(sar[:], sa[L][:, :, 0])
            # ACT counts: (S_k - S_{k+1}) / 2
            ca = cnt.tile([128, ACT_NB], F32, tag=f"ca{L}", name=f"ca{L}")
            nc.vector.tensor_tensor(ca[:], sar[:, 0:ACT_NB], sar[:, 1:ACT_NB+1],
                                    ALU.subtract)
            nc.vector.tensor_scalar(ca[:], ca[:], 0.5, None, ALU.mult)
            tt = tts[L]
            a0 = ACT_K_LO - K_LO
            pa2 = cnt.tile([128, ACT_NB], F32, tag=f"pa2{L}", name=f"pa2{L}")
            nc.vector.tensor_tensor(pa2[:], ca[:], tt[:, a0:a0+ACT_NB], ALU.mult)
            rsum2 = cnt.tile([128, 1], F32, tag=f"rs2{L}", name=f"rs2{L}")
            nc.vector.tensor_reduce(rsum2[:], pa2[:], mybir.AxisListType.X, ALU.add)
            if first:
                nc.vector.tensor_copy(acc[:], rsum2[:])
                first = False
            else:
                nc.vector.tensor_tensor(acc[:], acc[:], rsum2[:], ALU.add)
        nc.vector.tensor_scalar(acc[:], acc[:], -INV_LN2, None, ALU.mult)
        nc.sync.dma_start(bp.ap()[:], acc[:])
        # PE partial: per-slice PSUM reduces (overlap with trailing matmuls)
        pesl = cnt.tile([1, NSL], F32, tag="pesl", name="pesl")
        for s in range(NSL):
            nc.vector.tensor_reduce(pesl[:, s:s+1], psum[:, s*512:(s+1)*512],
                                    mybir.AxisListType.X, ALU.add)
        pes = cnt.tile([1, 1], F32, tag="pes", name="pes")
        nc.vector.tensor_reduce(pes[:], pesl[:], mybir.AxisListType.X, ALU.add)
        nc.vector.tensor_scalar(pes[:], pes[:], -INV_LN2, None, ALU.mult)
        nc.sync.dma_start(bq.ap()[:], pes[:])

    _split_multiwaits(nc)
    return nc


def _split_multiwaits(nc):
    """This walrus accepts at most 1 sync wait per instruction (2 for
    EventSemaphore); hoist extras onto preceding same-engine NoOps."""
    for f in nc.m.functions:
        for blk in f.blocks:
            il = blk.instructions
            i = 0
            while i < len(il):
                ins = il[i]
                si = ins.sync_info
                k = 2 if isinstance(ins, mybir.InstEventSemaphore) else 1
                if si is not None and len(si.on_wait) > k:
                    waits = list(si.on_wait)
                    extra, kept = waits[:-k], waits[-k:]
                    for w in extra:
                        nop = mybir.InstNoOp(
                            name=nc.get_next_instruction_name(), ins=[], outs=[])
                        nop.engine = ins.engine
                        nop.sync_info = mybir.SyncInfo(on_wait=[w], on_update=[])
                        il.insert(i, nop)
                        i += 1
                    si.on_wait = kept
                    ins.sync_info = si
                i += 1


def kernel(**inputs):
    if "nc" not in _CACHE:
        _CACHE["nc"] = build_nc()
    nc = _CACHE["nc"]

    x = np.ascontiguousarray(np.asarray(inputs["x"], np.float32))
    pp = _pack_params(inputs)
    kg = _CACHE.setdefault("kg", _kgrid())
    xr = x.reshape(B, C, NHW)
    in_maps = [{"xs": xr[i * B_PER_CORE:(i + 1) * B_PER_CORE],
                "pp": pp, "kg": kg} for i in range(N_CORES)]
    res = bass_utils.run_bass_kernel_spmd(nc, in_maps, core_ids=list(range(N_CORES)))
    out = np.empty((B, C, H, W), np.float32)
    bits = 0.0
    for i in range(N_CORES):
        out[i * B_PER_CORE:(i + 1) * B_PER_CORE] = \
            res.results[i]["ys"].reshape(B_PER_CORE, C, H, W)
        bits += float(res.results[i]["bp"].sum(dtype=np.float64))
        bits += float(res.results[i]["bq"].sum(dtype=np.float64))
    return out, np.float32(bits)


# revision 24
# speedup vs baseline: 1.1335x; 1.0673x over previous
"""Trainium2 Bass kernel for nn_EntropyBottleneck (8-core SPMD, data-parallel over N).

Math: values v = x transposed to [C, N]; outputs = round(v - m) + m (quantized!).
Because outputs are quantized, the per-channel MLP chain (1->3->3->3->3->1 with
tanh gates + sigmoid diff + log2) only takes ~21 distinct values per channel.
The kernel:
  1. builds the per-channel table T[c,k] = ln(likelihood(k + m_c)), k in
     [-10, 10], on device (tiny [128, 42] tiles; fully general math including
     softplus(matrix) = ln(1+exp(.)) and tanh(factor) gates),
  2. quantizes every element with the +1.5*2^23 magic-round trick (exact RNE,
     bitwise-matches jnp.round half-to-even); `outputs` is exact and unclamped,
  3. computes bits = sum T[c, r] by splitting the 21 bins across engines:
     - center bins [-2..2] (~79% of mass): ACT sign-fan accumulation
       (S_k = sum sign(r-k+0.5); counts = first differences; exact),
     - tail bins: DVE eq-masks (fp16, 4x mode) x TensorE ones... actually
       matmul with lhsT = per-channel T column (fp16), accumulating
       sum_c T[c,k]*mask_k[c,f] into PSUM across all bins/chunks; one final
       reduce yields the tail contribution directly (fp16 T rounding touches
       only the low-mass tails, ~1e-6 relative on bits),
  4. index clamp to [-10, 10]: boundary bins absorb out-of-range elements
     (~4e-6 relative bits error for 3-sigma-scaled inputs; outputs unaffected).
Host only shards/unshards and sums 8 partial bit sums.

Sharding: batch dim (16) split 2 per core across 8 cores; each core handles all
192 channels x 8192 samples.  Channels 0-127 on partitions (layout A); channels
128-191 duplicated across partition halves with the two batches split (layout B)
so every op uses all 128 partitions.

Measured on trn2 (8 cores): HW exec ~136 us; outputs bitwise-equal to the jax
reference, bits rel err ~4e-6.
"""
import sys
sys.path.insert(0, "/opt/trn_rl_repo")

import numpy as np
from contextlib import ExitStack

from concourse import bass, tile, bass_utils, mybir
from concourse.bass import Bass
from concourse.mybir import AluOpType as ALU, ActivationFunctionType as ACT

# ---------------- constants ----------------
B, C, H, W = 16, 192, 64, 64
NHW = H * W                      # 4096
N_CORES = 8
B_PER_CORE = B // N_CORES        # 2
MAGIC = 12582912.0               # 1.5 * 2^23: forces RNE round-to-int in f32
K_LO, K_HI = -10, 10
NBINS = K_HI - K_LO + 1          # 21
# bin split between engines: ACT takes the low contiguous range, DVE the rest
ACT_NB = 5                       # sign-fan bin count on ACT (center, exact)
ACT_K_LO = -2                    # ACT covers [-2 .. 2] (the high-mass center)
PE_BINS = [k for k in range(K_LO, K_HI + 1)
           if not (ACT_K_LO <= k < ACT_K_LO + ACT_NB)]   # tails via DVE-mask+PE
PE_NB = len(PE_BINS)
INV_LN2 = 1.4426950408889634
F32 = mybir.dt.float32
BF16 = mybir.dt.bfloat16
N_STAGES = 5

_CACHE = {}


def _pack_params(inputs):
    """[192, 59]: m0(3) m1(9) m2(9) m3(9) m4(3) | b0..b4(13) | f0..f3(12) | med(1)"""
    cols = []
    for i in range(N_STAGES):
        cols.append(np.asarray(inputs[f"matrix{i}"], np.float32).reshape(C, -1))
    for i in range(N_STAGES):
        cols.append(np.asarray(inputs[f"bias{i}"], np.float32).reshape(C, -1))
    for i in range(4):
        cols.append(np.asarray(inputs[f"factor{i}"], np.float32).reshape(C, -1))
    med = np.asarray(inputs["quantiles"], np.float32)[:, :, 1].reshape(C, 1)
    cols.append(med)
    return np.ascontiguousarray(np.concatenate(cols, axis=1))


# param column indices in the packed array
SP0 = 0            # matrix0: 3 cols
SP = [0, 3, 12, 21, 30]   # start col of matrix_i
BIA = [33, 36, 39, 42, 45]
FAC = 46           # factors: 12 cols (within tf tile: col i*3+j)
MED = 58


def _kgrid():
    ks = np.arange(K_LO, K_HI + 1, dtype=np.float32)
    sb = 0.5 - np.arange(ACT_K_LO, ACT_K_LO + ACT_NB + 1, dtype=np.float32)  # sign-fan biases
    g = np.concatenate([ks - 0.5, ks + 0.5, sb])      # [70 + ACT_NB + 1]
    return np.ascontiguousarray(np.broadcast_to(g, (128, g.size)).copy())


def _build_table(nc, pool, kgt, p, sp, tf):
    """Emit the per-channel MLP chain on [128, 70] tiles.  Returns Tt = ln(lh)."""
    W2 = 2 * NBINS

    def tl(nm):
        return pool.tile([128, W2], F32, tag=nm, name=nm)

    v = tl("tbl_v")
    nc.vector.tensor_scalar(v[:], kgt[:, 0:W2], p[:, MED:MED+1], None, ALU.add)
    h = [tl(f"tbl_h{j}") for j in range(3)]
    g = [tl(f"tbl_g{j}") for j in range(3)]
    tn = [tl(f"tbl_tn{j}") for j in range(3)]
    # stage 0: 1 -> 3, then tanh gate
    for j in range(3):
        nc.vector.tensor_scalar(h[j][:], v[:], sp[:, j:j+1],
                                p[:, BIA[0]+j:BIA[0]+j+1], ALU.mult, ALU.add)
        nc.scalar.activation(tn[j][:], h[j][:], ACT.Tanh)
        nc.vector.scalar_tensor_tensor(h[j][:], tn[j][:], tf[:, j:j+1], h[j][:],
                                       ALU.mult, ALU.add)
    # stages 1..3: 3 -> 3 with tanh gate
    for i in range(1, 4):
        for j in range(3):
            c0 = SP[i] + 3 * j
            nc.vector.tensor_scalar(g[j][:], h[0][:], sp[:, c0:c0+1],
                                    p[:, BIA[i]+j:BIA[i]+j+1], ALU.mult, ALU.add)
            nc.vector.scalar_tensor_tensor(g[j][:], h[1][:], sp[:, c0+1:c0+2],
                                           g[j][:], ALU.mult, ALU.add)
            nc.vector.scalar_tensor_tensor(g[j][:], h[2][:], sp[:, c0+2:c0+3],
                                           g[j][:], ALU.mult, ALU.add)
            nc.scalar.activation(tn[j][:], g[j][:], ACT.Tanh)
            nc.vector.scalar_tensor_tensor(g[j][:], tn[j][:], tf[:, 3*i+j:3*i+j+1],
                                           g[j][:], ALU.mult, ALU.add)
        h, g = g, h
    # stage 4: 3 -> 1
    o = tl("tbl_o")
    nc.vector.tensor_scalar(o[:], h[0][:], sp[:, SP[4]:SP[4]+1],
                            p[:, BIA[4]:BIA[4]+1], ALU.mult, ALU.add)
    nc.vector.scalar_tensor_tensor(o[:], h[1][:], sp[:, SP[4]+1:SP[4]+2], o[:],
                                   ALU.mult, ALU.add)
    nc.vector.scalar_tensor_tensor(o[:], h[2][:], sp[:, SP[4]+2:SP[4]+3], o[:],
                                   ALU.mult, ALU.add)
    lo, up = o[:, 0:NBINS], o[:, NBINS:W2]
    t1 = pool.tile([128, NBINS], F32)
    nc.vector.tensor_tensor(t1[:], lo, up, ALU.add)
    sg = pool.tile([128, NBINS], F32)
    nc.scalar.activation(sg[:], t1[:], ACT.Sign, scale=-1.0)   # -sign(lo+up)
    su = pool.tile([128, NBINS], F32)
    sl = pool.tile([128, NBINS], F32)
    nc.vector.tensor_tensor(su[:], up, sg[:], ALU.mult)
    nc.vector.tensor_tensor(sl[:], lo, sg[:], ALU.mult)
    nc.scalar.activation(su[:], su[:], ACT.Sigmoid)
    nc.scalar.activation(sl[:], sl[:], ACT.Sigmoid)
    d = pool.tile([128, NBINS], F32)
    nc.vector.tensor_tensor(d[:], su[:], sl[:], ALU.subtract)
    nc.scalar.activation(d[:], d[:], ACT.Abs)
    nc.vector.tensor_scalar(d[:], d[:], 1e-9, None, ALU.max)
    tt = pool.tile([128, NBINS], F32)
    nc.scalar.activation(tt[:], d[:], ACT.Ln)
    return tt


def build_nc():
    nc = Bass("TRN2", target_bir_lowering=False, debug=False,
              enable_asserts=False, num_devices=N_CORES)
    xs = nc.dram_tensor("xs", [B_PER_CORE, C, NHW], F32, kind="ExternalInput")
    pp = nc.dram_tensor("pp", [C, 59], F32, kind="ExternalInput")
    kg = nc.dram_tensor("kg", [128, 2 * NBINS + ACT_NB + 1], F32, kind="ExternalInput")
    ys = nc.dram_tensor("ys", [B_PER_CORE, C, NHW], F32, kind="ExternalOutput")
    bp = nc.dram_tensor("bp", [128, 1], F32, kind="ExternalOutput")
    bq = nc.dram_tensor("bq", [1, 1], F32, kind="ExternalOutput")

    with ExitStack() as ctx:
        tc = ctx.enter_context(tile.TileContext(nc))
        prm = ctx.enter_context(tc.tile_pool(name="prm", bufs=1))
        tbl = ctx.enter_context(tc.tile_pool(name="tbl", bufs=2))
        xp = ctx.enter_context(tc.tile_pool(name="xp", bufs=2))
        tp = ctx.enter_context(tc.tile_pool(name="tp", bufs=2))
        rp = ctx.enter_context(tc.tile_pool(name="rp", bufs=2))
        scr = ctx.enter_context(tc.tile_pool(name="scr", bufs=1))
        cnt = ctx.enter_context(tc.tile_pool(name="cnt", bufs=1))
        wmp = ctx.enter_context(tc.tile_pool(name="wmp", bufs=2))
        psp = ctx.enter_context(tc.tile_pool(name="psp", bufs=1, space="PSUM"))

        # ---- parameters ----
        pa = prm.tile([128, 59], F32, tag="pa", name="pa")
        nc.sync.dma_start(pa[:], pp.ap()[0:128, :])
        pb = prm.tile([128, 59], F32, tag="pb", name="pb")
        nc.sync.dma_start(pb[0:64, :], pp.ap()[128:192, :])
        nc.sync.dma_start(pb[64:128, :], pp.ap()[128:192, :])
        kgt = prm.tile([128, 2 * NBINS + ACT_NB + 1], F32, tag="kgt", name="kgt")
        nc.sync.dma_start(kgt[:], kg.ap()[:])

        spt, tft, tts, t16 = {}, {}, {}, {}
        omb = {}   # per-layout outp bias col: med - MAGIC
        for L, p in (("A", pa), ("B", pb)):
            ob = prm.tile([128, 1], F32, tag=f"omb{L}", name=f"omb{L}")
            nc.vector.tensor_scalar(ob[:], p[:, MED:MED+1], MAGIC, None, ALU.subtract)
            omb[L] = ob
            s = prm.tile([128, 33], F32, tag=f"sp{L}", name=f"sp{L}")
            # softplus(x) = ln(1 + exp(x)) — no Softplus in the ACT tables here
            nc.scalar.activation(s[:], p[:, 0:33], ACT.Exp)
            nc.vector.tensor_scalar(s[:], s[:], 1.0, None, ALU.add)
            nc.scalar.activation(s[:], s[:], ACT.Ln)
            f = prm.tile([128, 12], F32, tag=f"tf{L}", name=f"tf{L}")
            nc.scalar.activation(f[:], p[:, FAC:FAC+12], ACT.Tanh)
            spt[L], tft[L] = s, f

        # ---- tables (tiny) ----
        for L, p in (("A", pa), ("B", pb)):
            tts[L] = _build_table(nc, tbl, kgt, p, spt[L], tft[L])
            w16 = prm.tile([128, NBINS], mybir.dt.float16, tag=f"t16{L}", name=f"t16{L}")
            nc.vector.tensor_copy(w16[:], tts[L][:])
            t16[L] = w16

        # ---- count accumulators ----
        # ACT sign sums: [128, ACT_NB+1, nchunks]; PE bins accumulate in PSUM
        chunks = [("A", 0), ("A", 1), ("B", None)]
        nch = {"A": 2, "B": 1}
        sa = {L: cnt.tile([128, ACT_NB + 1, nch[L]], F32, tag=f"sa{L}", name=f"sa{L}") for L in "AB"}
        psum = psp.tile([1, NHW], F32, tag="psum", name="psum")
        NSL = NHW // 512

        # ---- main loop ----
        for (L, b) in chunks:
            p = pa if L == "A" else pb
            med = p[:, MED:MED+1]
            x = xp.tile([128, NHW], F32, tag="x", name="x")
            if L == "A":
                nc.sync.dma_start(x[:], xs.ap()[b, 0:128, :])
            else:
                nc.sync.dma_start(x[0:64, :], xs.ap()[0, 128:192, :])
                nc.sync.dma_start(x[64:128, :], xs.ap()[1, 128:192, :])
            t = tp.tile([128, NHW], F32, tag="t", name="t")
            # t = (x - m) + MAGIC  -> integer-valued + MAGIC (exact RNE round)
            nc.vector.tensor_scalar(t[:], x[:], med, MAGIC, ALU.subtract, ALU.add)
            # outputs = t + (m - MAGIC), on ACT to offload DVE
            nc.scalar.activation(x[:], t[:], ACT.Identity, bias=omb[L][:, 0:1])
            if L == "A":
                nc.sync.dma_start(ys.ap()[b, 0:128, :], x[:])
            else:
                nc.sync.dma_start(ys.ap()[0, 128:192, :], x[0:64, :])
                nc.sync.dma_start(ys.ap()[1, 128:192, :], x[64:128, :])
            # r (bf16, clamped) for binning
            r = rp.tile([128, NHW], BF16, tag="r", name="r")
            nc.vector.tensor_scalar(r[:], t[:], MAGIC, None, ALU.subtract)
            rc = rp.tile([128, NHW], BF16, tag="rc", name="rc")
            nc.vector.tensor_scalar(rc[:], r[:], float(K_LO), float(K_HI),
                                    ALU.max, ALU.min)
            ci = b if L == "A" else 0
            # PE bins: T-weighted masks on DVE, summed over channels by PE
            tt = tts[L]
            first_chunk = (L == "A" and b == 0)
            last_chunk = (L == "B")
            w16L = t16[L]
            for i, k in enumerate(PE_BINS):
                kcol = k - K_LO
                wm = wmp.tile([128, NHW], mybir.dt.float16, tag="wm", name="wm")
                nc.vector.tensor_scalar(wm[:], rc[:], float(k), None, ALU.is_equal)
                for s in range(NSL):
                    nc.tensor.matmul(psum[:, s*512:(s+1)*512],
                                     w16L[:, kcol:kcol+1], wm[:, s*512:(s+1)*512],
                                     start=(first_chunk and i == 0),
                                     stop=(last_chunk and i == PE_NB - 1))
            # ACT bins: sign-fan partial sums  S_k = sum sign(r - (k-0.5))
            sact = scr.tile([128, NHW], BF16, tag="sact", name="sact")
            for i in range(ACT_NB + 1):
                k = K_LO + i
                nc.scalar.activation(sact[:], rc[:], ACT.Sign,
                                     bias=kgt[:, 2*NBINS+i:2*NBINS+i+1],
                                     accum_out=sa[L][:, i, ci:ci+1])

        # ---- finalize bits ----
        acc = cnt.tile([128, 1], F32, tag="acc", name="acc")
        first = True
        for L in ("A", "B"):
            sar = cnt.tile([128, ACT_NB + 1], F32, tag=f"sar{L}", name=f"sar{L}")
            if nch[L] > 1:
                nc.vector.tensor_reduce(sar[:], sa[L][:], mybir.AxisListType.X, ALU.add)
            else:
                nc.vector.tensor_copy(sar[:], sa[L][:, :, 0])
            # ACT counts: (S_k - S_{k+1}) / 2
            ca = cnt.tile([128, ACT_NB], F32, tag=f"ca{L}", name=f"ca{L}")
            nc.vector.tensor_tensor(ca[:], sar[:, 0:ACT_NB], sar[:, 1:ACT_NB+1],
                                    ALU.subtract)
            nc.vector.tensor_scalar(ca[:], ca[:], 0.5, None, ALU.mult)
            tt = tts[L]
            a0 = ACT_K_LO - K_LO
            pa2 = cnt.tile([128, ACT_NB], F32, tag=f"pa2{L}", name=f"pa2{L}")
            nc.vector.tensor_tensor(pa2[:], ca[:], tt[:, a0:a0+ACT_NB], ALU.mult)
            rsum2 = cnt.tile([128, 1], F32, tag=f"rs2{L}", name=f"rs2{L}")
            nc.vector.tensor_reduce(rsum2[:], pa2[:], mybir.AxisListType.X, ALU.add)
            if first:
                nc.vector.tensor_copy(acc[:], rsum2[:])
                first = False
            else:
                nc.vector.tensor_tensor(acc[:], acc[:], rsum2[:], ALU.add)
        nc.vector.tensor_scalar(acc[:], acc[:], -INV_LN2, None, ALU.mult)
        nc.sync.dma_start(bp.ap()[:], acc[:])
        # PE partial: per-slice PSUM reduces (overlap with trailing matmuls)
        pesl = cnt.tile([1, NSL], F32, tag="pesl", name="pesl")
        for s in range(NSL):
            nc.vector.tensor_reduce(pesl[:, s:s+1], psum[:, s*512:(s+1)*512],
                                    mybir.AxisListType.X, ALU.add)
        pes = cnt.tile([1, 1], F32, tag="pes", name="pes")
        nc.vector.tensor_reduce(pes[:], pesl[:], mybir.AxisListType.X, ALU.add)
        nc.vector.tensor_scalar(pes[:], pes[:], -INV_LN2, None, ALU.mult)
        nc.sync.dma_start(bq.ap()[:], pes[:])

    _split_multiwaits(nc)
    return nc


def _split_multiwaits(nc):
    """This walrus accepts at most 1 sync wait per instruction (2 for
    EventSemaphore); hoist extras onto preceding same-engine NoOps."""
    for f in nc.m.functions:
        for blk in f.blocks:
            il = blk.instructions
            i = 0
            while i < len(il):
                ins = il[i]
                si = ins.sync_info
                k = 2 if isinstance(ins, mybir.InstEventSemaphore) else 1
                if si is not None and len(si.on_wait) > k:
                    waits = list(si.on_wait)
                    extra, kept = waits[:-k], waits[-k:]
                    for w in extra:
                        nop = mybir.InstNoOp(
                            name=nc.get_next_instruction_name(), ins=[], outs=[])
                        nop.engine = ins.engine
                        nop.sync_info = mybir.SyncInfo(on_wait=[w], on_update=[])
                        il.insert(i, nop)
                        i += 1
                    si.on_wait = kept
                    ins.sync_info = si
                i += 1


def kernel(**inputs):
    if "nc" not in _CACHE:
        _CACHE["nc"] = build_nc()
    nc = _CACHE["nc"]

    x = np.ascontiguousarray(np.asarray(inputs["x"], np.float32))
    pp = _pack_params(inputs)
    kg = _CACHE.setdefault("kg", _kgrid())
    xr = x.reshape(B, C, NHW)
    in_maps = [{"xs": xr[i * B_PER_CORE:(i + 1) * B_PER_CORE],
                "pp": pp, "kg": kg} for i in range(N_CORES)]
    res = bass_utils.run_bass_kernel_spmd(nc, in_maps, core_ids=list(range(N_CORES)))
    out = np.empty((B, C, H, W), np.float32)
    bits = 0.0
    for i in range(N_CORES):
        out[i * B_PER_CORE:(i + 1) * B_PER_CORE] = \
            res.results[i]["ys"].reshape(B_PER_CORE, C, H, W)
        bits += float(res.results[i]["bp"].sum(dtype=np.float64))
        bits += float(res.results[i]["bq"].sum(dtype=np.float64))
    return out, np.float32(bits)


# revision 25
# speedup vs baseline: 1.1691x; 1.0314x over previous
"""Trainium2 Bass kernel for nn_EntropyBottleneck (8-core SPMD, data-parallel over N).

Math: values v = x transposed to [C, N]; outputs = round(v - m) + m (quantized!).
Because outputs are quantized, the per-channel MLP chain (1->3->3->3->3->1 with
tanh gates + sigmoid diff + log2) only takes ~21 distinct values per channel.
The kernel:
  1. builds the per-channel table T[c,k] = ln(likelihood(k + m_c)), k in
     [-10, 10], on device (tiny [128, 42] tiles; fully general math including
     softplus(matrix) = ln(1+exp(.)) and tanh(factor) gates),
  2. quantizes every element with the +1.5*2^23 magic-round trick (exact RNE,
     bitwise-matches jnp.round half-to-even); `outputs` is exact and unclamped,
  3. computes bits = sum T[c, r] by splitting the 21 bins across engines:
     - center bins [-2..2] (~79% of mass): ACT sign-fan accumulation
       (S_k = sum sign(r-k+0.5); counts = first differences; exact),
     - tail bins: DVE eq-masks (fp16, 4x mode) x TensorE ones... actually
       matmul with lhsT = per-channel T column (fp16), accumulating
       sum_c T[c,k]*mask_k[c,f] into PSUM across all bins/chunks; one final
       reduce yields the tail contribution directly (fp16 T rounding touches
       only the low-mass tails, ~1e-6 relative on bits),
  4. index clamp to [-10, 10]: boundary bins absorb out-of-range elements
     (~4e-6 relative bits error for 3-sigma-scaled inputs; outputs unaffected).
Host only shards/unshards and sums 8 partial bit sums.

Sharding: batch dim (16) split 2 per core across 8 cores; each core handles all
192 channels x 8192 samples.  Channels 0-127 on partitions (layout A); channels
128-191 duplicated across partition halves with the two batches split (layout B)
so every op uses all 128 partitions.

Measured on trn2 (8 cores): HW exec ~136 us; outputs bitwise-equal to the jax
reference, bits rel err ~4e-6.
"""
import sys
sys.path.insert(0, "/opt/trn_rl_repo")

import numpy as np
from contextlib import ExitStack

from concourse import bass, tile, bass_utils, mybir
from concourse.bass import Bass
from concourse.mybir import AluOpType as ALU, ActivationFunctionType as ACT

# ---------------- constants ----------------
B, C, H, W = 16, 192, 64, 64
NHW = H * W                      # 4096
N_CORES = 8
B_PER_CORE = B // N_CORES        # 2
MAGIC = 12582912.0               # 1.5 * 2^23: forces RNE round-to-int in f32
K_LO, K_HI = -10, 10
NBINS = K_HI - K_LO + 1          # 21
# bin split between engines: ACT takes the low contiguous range, DVE the rest
ACT_NB = 5                       # sign-fan bin count on ACT (center, exact)
ACT_K_LO = -2                    # ACT covers [-2 .. 2] (the high-mass center)
PE_BINS = [k for k in range(K_LO, K_HI + 1)
           if not (ACT_K_LO <= k < ACT_K_LO + ACT_NB)]   # tails via DVE-mask+PE
PE_NB = len(PE_BINS)
INV_LN2 = 1.4426950408889634
F32 = mybir.dt.float32
BF16 = mybir.dt.bfloat16
N_STAGES = 5

_CACHE = {}


def _pack_params(inputs):
    """[192, 59]: m0(3) m1(9) m2(9) m3(9) m4(3) | b0..b4(13) | f0..f3(12) | med(1)"""
    cols = []
    for i in range(N_STAGES):
        cols.append(np.asarray(inputs[f"matrix{i}"], np.float32).reshape(C, -1))
    for i in range(N_STAGES):
        cols.append(np.asarray(inputs[f"bias{i}"], np.float32).reshape(C, -1))
    for i in range(4):
        cols.append(np.asarray(inputs[f"factor{i}"], np.float32).reshape(C, -1))
    med = np.asarray(inputs["quantiles"], np.float32)[:, :, 1].reshape(C, 1)
    cols.append(med)
    return np.ascontiguousarray(np.concatenate(cols, axis=1))


# param column indices in the packed array
SP0 = 0            # matrix0: 3 cols
SP = [0, 3, 12, 21, 30]   # start col of matrix_i
BIA = [33, 36, 39, 42, 45]
FAC = 46           # factors: 12 cols (within tf tile: col i*3+j)
MED = 58


def _kgrid():
    ks = np.arange(K_LO, K_HI + 1, dtype=np.float32)
    sb = 0.5 - np.arange(ACT_K_LO, ACT_K_LO + ACT_NB + 1, dtype=np.float32)  # sign-fan biases
    g = np.concatenate([ks - 0.5, ks + 0.5, sb])      # [70 + ACT_NB + 1]
    return np.ascontiguousarray(np.broadcast_to(g, (128, g.size)).copy())


def _build_table(nc, pool, kgt, p, sp, tf):
    """Emit the per-channel MLP chain on [128, 70] tiles.  Returns Tt = ln(lh)."""
    W2 = 2 * NBINS

    def tl(nm):
        return pool.tile([128, W2], F32, tag=nm, name=nm)

    v = tl("tbl_v")
    nc.vector.tensor_scalar(v[:], kgt[:, 0:W2], p[:, MED:MED+1], None, ALU.add)
    h = [tl(f"tbl_h{j}") for j in range(3)]
    g = [tl(f"tbl_g{j}") for j in range(3)]
    tn = [tl(f"tbl_tn{j}") for j in range(3)]
    # stage 0: 1 -> 3, then tanh gate
    for j in range(3):
        nc.vector.tensor_scalar(h[j][:], v[:], sp[:, j:j+1],
                                p[:, BIA[0]+j:BIA[0]+j+1], ALU.mult, ALU.add)
        nc.scalar.activation(tn[j][:], h[j][:], ACT.Tanh)
        nc.vector.scalar_tensor_tensor(h[j][:], tn[j][:], tf[:, j:j+1], h[j][:],
                                       ALU.mult, ALU.add)
    # stages 1..3: 3 -> 3 with tanh gate
    for i in range(1, 4):
        for j in range(3):
            c0 = SP[i] + 3 * j
            nc.vector.tensor_scalar(g[j][:], h[0][:], sp[:, c0:c0+1],
                                    p[:, BIA[i]+j:BIA[i]+j+1], ALU.mult, ALU.add)
            nc.vector.scalar_tensor_tensor(g[j][:], h[1][:], sp[:, c0+1:c0+2],
                                           g[j][:], ALU.mult, ALU.add)
            nc.vector.scalar_tensor_tensor(g[j][:], h[2][:], sp[:, c0+2:c0+3],
                                           g[j][:], ALU.mult, ALU.add)
            nc.scalar.activation(tn[j][:], g[j][:], ACT.Tanh)
            nc.vector.scalar_tensor_tensor(g[j][:], tn[j][:], tf[:, 3*i+j:3*i+j+1],
                                           g[j][:], ALU.mult, ALU.add)
        h, g = g, h
    # stage 4: 3 -> 1
    o = tl("tbl_o")
    nc.vector.tensor_scalar(o[:], h[0][:], sp[:, SP[4]:SP[4]+1],
                            p[:, BIA[4]:BIA[4]+1], ALU.mult, ALU.add)
    nc.vector.scalar_tensor_tensor(o[:], h[1][:], sp[:, SP[4]+1:SP[4]+2], o[:],
                                   ALU.mult, ALU.add)
    nc.vector.scalar_tensor_tensor(o[:], h[2][:], sp[:, SP[4]+2:SP[4]+3], o[:],
                                   ALU.mult, ALU.add)
    lo, up = o[:, 0:NBINS], o[:, NBINS:W2]
    t1 = pool.tile([128, NBINS], F32)
    nc.vector.tensor_tensor(t1[:], lo, up, ALU.add)
    sg = pool.tile([128, NBINS], F32)
    nc.scalar.activation(sg[:], t1[:], ACT.Sign, scale=-1.0)   # -sign(lo+up)
    su = pool.tile([128, NBINS], F32)
    sl = pool.tile([128, NBINS], F32)
    nc.vector.tensor_tensor(su[:], up, sg[:], ALU.mult)
    nc.vector.tensor_tensor(sl[:], lo, sg[:], ALU.mult)
    nc.scalar.activation(su[:], su[:], ACT.Sigmoid)
    nc.scalar.activation(sl[:], sl[:], ACT.Sigmoid)
    d = pool.tile([128, NBINS], F32)
    nc.vector.tensor_tensor(d[:], su[:], sl[:], ALU.subtract)
    nc.scalar.activation(d[:], d[:], ACT.Abs)
    nc.vector.tensor_scalar(d[:], d[:], 1e-9, None, ALU.max)
    tt = pool.tile([128, NBINS], F32)
    nc.scalar.activation(tt[:], d[:], ACT.Ln)
    return tt


def build_nc():
    nc = Bass("TRN2", target_bir_lowering=False, debug=False,
              enable_asserts=False, num_devices=N_CORES)
    xs = nc.dram_tensor("xs", [B_PER_CORE, C, NHW], F32, kind="ExternalInput")
    pp = nc.dram_tensor("pp", [C, 59], F32, kind="ExternalInput")
    kg = nc.dram_tensor("kg", [128, 2 * NBINS + ACT_NB + 1], F32, kind="ExternalInput")
    ys = nc.dram_tensor("ys", [B_PER_CORE, C, NHW], F32, kind="ExternalOutput")
    bp = nc.dram_tensor("bp", [128, 1], F32, kind="ExternalOutput")
    bq = nc.dram_tensor("bq", [1, 1], F32, kind="ExternalOutput")

    with ExitStack() as ctx:
        tc = ctx.enter_context(tile.TileContext(nc))
        prm = ctx.enter_context(tc.tile_pool(name="prm", bufs=1))
        tbl = ctx.enter_context(tc.tile_pool(name="tbl", bufs=2))
        xp = ctx.enter_context(tc.tile_pool(name="xp", bufs=2))
        tp = ctx.enter_context(tc.tile_pool(name="tp", bufs=2))
        rp = ctx.enter_context(tc.tile_pool(name="rp", bufs=2))
        scr = ctx.enter_context(tc.tile_pool(name="scr", bufs=1))
        cnt = ctx.enter_context(tc.tile_pool(name="cnt", bufs=1))
        wmp = ctx.enter_context(tc.tile_pool(name="wmp", bufs=4))
        psp = ctx.enter_context(tc.tile_pool(name="psp", bufs=1, space="PSUM"))

        # ---- parameters ----
        pa = prm.tile([128, 59], F32, tag="pa", name="pa")
        nc.sync.dma_start(pa[:], pp.ap()[0:128, :])
        pb = prm.tile([128, 59], F32, tag="pb", name="pb")
        nc.sync.dma_start(pb[0:64, :], pp.ap()[128:192, :])
        nc.sync.dma_start(pb[64:128, :], pp.ap()[128:192, :])
        kgt = prm.tile([128, 2 * NBINS + ACT_NB + 1], F32, tag="kgt", name="kgt")
        nc.sync.dma_start(kgt[:], kg.ap()[:])

        spt, tft, tts, t16 = {}, {}, {}, {}
        omb = {}   # per-layout outp bias col: med - MAGIC
        for L, p in (("A", pa), ("B", pb)):
            ob = prm.tile([128, 1], F32, tag=f"omb{L}", name=f"omb{L}")
            nc.vector.tensor_scalar(ob[:], p[:, MED:MED+1], MAGIC, None, ALU.subtract)
            omb[L] = ob
            s = prm.tile([128, 33], F32, tag=f"sp{L}", name=f"sp{L}")
            # softplus(x) = ln(1 + exp(x)) — no Softplus in the ACT tables here
            nc.scalar.activation(s[:], p[:, 0:33], ACT.Exp)
            nc.vector.tensor_scalar(s[:], s[:], 1.0, None, ALU.add)
            nc.scalar.activation(s[:], s[:], ACT.Ln)
            f = prm.tile([128, 12], F32, tag=f"tf{L}", name=f"tf{L}")
            nc.scalar.activation(f[:], p[:, FAC:FAC+12], ACT.Tanh)
            spt[L], tft[L] = s, f

        # ---- tables (tiny) ----
        for L, p in (("A", pa), ("B", pb)):
            tts[L] = _build_table(nc, tbl, kgt, p, spt[L], tft[L])
            w16 = prm.tile([128, NBINS], mybir.dt.float16, tag=f"t16{L}", name=f"t16{L}")
            nc.vector.tensor_copy(w16[:], tts[L][:])
            t16[L] = w16

        # ---- count accumulators ----
        # ACT sign sums: [128, ACT_NB+1, nchunks]; PE bins accumulate in PSUM
        chunks = [("A", 0), ("A", 1), ("B", None)]
        nch = {"A": 2, "B": 1}
        sa = {L: cnt.tile([128, ACT_NB + 1, nch[L]], F32, tag=f"sa{L}", name=f"sa{L}") for L in "AB"}
        psum = psp.tile([1, NHW], F32, tag="psum", name="psum")
        NSL = NHW // 512

        # ---- main loop ----
        for (L, b) in chunks:
            p = pa if L == "A" else pb
            med = p[:, MED:MED+1]
            x = xp.tile([128, NHW], F32, tag="x", name="x")
            if L == "A":
                nc.sync.dma_start(x[:], xs.ap()[b, 0:128, :])
            else:
                nc.sync.dma_start(x[0:64, :], xs.ap()[0, 128:192, :])
                nc.sync.dma_start(x[64:128, :], xs.ap()[1, 128:192, :])
            t = tp.tile([128, NHW], F32, tag="t", name="t")
            # t = (x - m) + MAGIC  -> integer-valued + MAGIC (exact RNE round)
            nc.vector.tensor_scalar(t[:], x[:], med, MAGIC, ALU.subtract, ALU.add)
            # outputs = t + (m - MAGIC), on ACT to offload DVE
            nc.scalar.activation(x[:], t[:], ACT.Identity, bias=omb[L][:, 0:1])
            if L == "A":
                nc.sync.dma_start(ys.ap()[b, 0:128, :], x[:])
            else:
                nc.sync.dma_start(ys.ap()[0, 128:192, :], x[0:64, :])
                nc.sync.dma_start(ys.ap()[1, 128:192, :], x[64:128, :])
            # r (bf16, clamped) for binning
            r = rp.tile([128, NHW], BF16, tag="r", name="r")
            nc.vector.tensor_scalar(r[:], t[:], MAGIC, None, ALU.subtract)
            rc = rp.tile([128, NHW], BF16, tag="rc", name="rc")
            nc.vector.tensor_scalar(rc[:], r[:], float(K_LO), float(K_HI),
                                    ALU.max, ALU.min)
            ci = b if L == "A" else 0
            # PE bins: T-weighted masks on DVE, summed over channels by PE
            tt = tts[L]
            first_chunk = (L == "A" and b == 0)
            last_chunk = (L == "B")
            w16L = t16[L]
            for i, k in enumerate(PE_BINS):
                kcol = k - K_LO
                wm = wmp.tile([128, NHW], mybir.dt.float16, tag="wm", name="wm")
                nc.vector.tensor_scalar(wm[:], rc[:], float(k), None, ALU.is_equal)
                for s in range(NSL):
                    nc.tensor.matmul(psum[:, s*512:(s+1)*512],
                                     w16L[:, kcol:kcol+1], wm[:, s*512:(s+1)*512],
                                     start=(first_chunk and i == 0),
                                     stop=(last_chunk and i == PE_NB - 1))
            # ACT bins: sign-fan partial sums  S_k = sum sign(r - (k-0.5))
            sact = scr.tile([128, NHW], BF16, tag="sact", name="sact")
            for i in range(ACT_NB + 1):
                k = K_LO + i
                nc.scalar.activation(sact[:], rc[:], ACT.Sign,
                                     bias=kgt[:, 2*NBINS+i:2*NBINS+i+1],
                                     accum_out=sa[L][:, i, ci:ci+1])

        # ---- finalize bits ----
        acc = cnt.tile([128, 1], F32, tag="acc", name="acc")
        first = True
        for L in ("A", "B"):
            sar = cnt.tile([128, ACT_NB + 1], F32, tag=f"sar{L}", name=f"sar{L}")
            if nch[L] > 1:
                nc.vector.tensor_reduce(sar[:], sa[L][:], mybir.AxisListType.X, ALU.add)
            else:
                nc.vector.tensor_copy(sar[:], sa[L][:, :, 0])
            # ACT counts: (S_k - S_{k+1}) / 2
            ca = cnt.tile([128, ACT_NB], F32, tag=f"ca{L}", name=f"ca{L}")
            nc.vector.tensor_tensor(ca[:], sar[:, 0:ACT_NB], sar[:, 1:ACT_NB+1],
                                    ALU.subtract)
            nc.vector.tensor_scalar(ca[:], ca[:], 0.5, None, ALU.mult)
            tt = tts[L]
            a0 = ACT_K_LO - K_LO
            pa2 = cnt.tile([128, ACT_NB], F32, tag=f"pa2{L}", name=f"pa2{L}")
            nc.vector.tensor_tensor(pa2[:], ca[:], tt[:, a0:a0+ACT_NB], ALU.mult)
            rsum2 = cnt.tile([128, 1], F32, tag=f"rs2{L}", name=f"rs2{L}")
            nc.vector.tensor_reduce(rsum2[:], pa2[:], mybir.AxisListType.X, ALU.add)
            if first:
                nc.vector.tensor_copy(acc[:], rsum2[:])
                first = False
            else:
                nc.vector.tensor_tensor(acc[:], acc[:], rsum2[:], ALU.add)
        nc.vector.tensor_scalar(acc[:], acc[:], -INV_LN2, None, ALU.mult)
        nc.sync.dma_start(bp.ap()[:], acc[:])
        # PE partial: per-slice PSUM reduces (overlap with trailing matmuls)
        pesl = cnt.tile([1, NSL], F32, tag="pesl", name="pesl")
        for s in range(NSL):
            nc.vector.tensor_reduce(pesl[:, s:s+1], psum[:, s*512:(s+1)*512],
                                    mybir.AxisListType.X, ALU.add)
        pes = cnt.tile([1, 1], F32, tag="pes", name="pes")
        nc.vector.tensor_reduce(pes[:], pesl[:], mybir.AxisListType.X, ALU.add)
        nc.vector.tensor_scalar(pes[:], pes[:], -INV_LN2, None, ALU.mult)
        nc.sync.dma_start(bq.ap()[:], pes[:])

    _split_multiwaits(nc)
    return nc


def _split_multiwaits(nc):
    """This walrus accepts at most 1 sync wait per instruction (2 for
    EventSemaphore); hoist extras onto preceding same-engine NoOps."""
    for f in nc.m.functions:
        for blk in f.blocks:
            il = blk.instructions
            i = 0
            while i < len(il):
                ins = il[i]
                si = ins.sync_info
                k = 2 if isinstance(ins, mybir.InstEventSemaphore) else 1
                if si is not None and len(si.on_wait) > k:
                    waits = list(si.on_wait)
                    extra, kept = waits[:-k], waits[-k:]
                    for w in extra:
                        nop = mybir.InstNoOp(
                            name=nc.get_next_instruction_name(), ins=[], outs=[])
                        nop.engine = ins.engine
                        nop.sync_info = mybir.SyncInfo(on_wait=[w], on_update=[])
                        il.insert(i, nop)
                        i += 1
                    si.on_wait = kept
                    ins.sync_info = si
                i += 1


def kernel(**inputs):
    if "nc" not in _CACHE:
        _CACHE["nc"] = build_nc()
    nc = _CACHE["nc"]

    x = np.ascontiguousarray(np.asarray(inputs["x"], np.float32))
    pp = _pack_params(inputs)
    kg = _CACHE.setdefault("kg", _kgrid())
    xr = x.reshape(B, C, NHW)
    in_maps = [{"xs": xr[i * B_PER_CORE:(i + 1) * B_PER_CORE],
                "pp": pp, "kg": kg} for i in range(N_CORES)]
    res = bass_utils.run_bass_kernel_spmd(nc, in_maps, core_ids=list(range(N_CORES)))
    out = np.empty((B, C, H, W), np.float32)
    bits = 0.0
    for i in range(N_CORES):
        out[i * B_PER_CORE:(i + 1) * B_PER_CORE] = \
            res.results[i]["ys"].reshape(B_PER_CORE, C, H, W)
        bits += float(res.results[i]["bp"].sum(dtype=np.float64))
        bits += float(res.results[i]["bq"].sum(dtype=np.float64))
    return out, np.float32(bits)


# revision 26
# speedup vs baseline: 1.1787x; 1.0082x over previous
"""Trainium2 Bass kernel for nn_EntropyBottleneck (8-core SPMD, data-parallel over N).

Math: values v = x transposed to [C, N]; outputs = round(v - m) + m (quantized!).
Because outputs are quantized, the per-channel MLP chain (1->3->3->3->3->1 with
tanh gates + sigmoid diff + log2) only takes ~21 distinct values per channel.
The kernel:
  1. builds the per-channel table T[c,k] = ln(likelihood(k + m_c)), k in
     [-10, 10], on device (tiny [128, 42] tiles; fully general math including
     softplus(matrix) = ln(1+exp(.)) and tanh(factor) gates),
  2. quantizes every element with the +1.5*2^23 magic-round trick (exact RNE,
     bitwise-matches jnp.round half-to-even); `outputs` is exact and unclamped,
  3. computes bits = sum T[c, r] by splitting the 21 bins across engines:
     - center bins [-2..2] (~79% of mass): ACT sign-fan accumulation
       (S_k = sum sign(r-k+0.5); counts = first differences; exact),
     - tail bins: DVE eq-masks (fp16, 4x mode) x TensorE ones... actually
       matmul with lhsT = per-channel T column (fp16), accumulating
       sum_c T[c,k]*mask_k[c,f] into PSUM across all bins/chunks; one final
       reduce yields the tail contribution directly (fp16 T rounding touches
       only the low-mass tails, ~1e-6 relative on bits),
  4. index clamp to [-10, 10]: boundary bins absorb out-of-range elements
     (~4e-6 relative bits error for 3-sigma-scaled inputs; outputs unaffected).
Host only shards/unshards and sums 8 partial bit sums.

Sharding: batch dim (16) split 2 per core across 8 cores; each core handles all
192 channels x 8192 samples.  Channels 0-127 on partitions (layout A); channels
128-191 duplicated across partition halves with the two batches split (layout B)
so every op uses all 128 partitions.

Measured on trn2 (8 cores): HW exec ~136 us; outputs bitwise-equal to the jax
reference, bits rel err ~4e-6.
"""
import sys
sys.path.insert(0, "/opt/trn_rl_repo")

import numpy as np
from contextlib import ExitStack

from concourse import bass, tile, bass_utils, mybir
from concourse.bass import Bass
from concourse.mybir import AluOpType as ALU, ActivationFunctionType as ACT

# ---------------- constants ----------------
B, C, H, W = 16, 192, 64, 64
NHW = H * W                      # 4096
N_CORES = 8
B_PER_CORE = B // N_CORES        # 2
MAGIC = 12582912.0               # 1.5 * 2^23: forces RNE round-to-int in f32
K_LO, K_HI = -10, 10
NBINS = K_HI - K_LO + 1          # 21
# bin split between engines: ACT takes the low contiguous range, DVE the rest
ACT_NB = 5                       # sign-fan bin count on ACT (center, exact)
ACT_K_LO = -2                    # ACT covers [-2 .. 2] (the high-mass center)
PE_BINS = [k for k in range(K_LO, K_HI + 1)
           if not (ACT_K_LO <= k < ACT_K_LO + ACT_NB)]   # tails via DVE-mask+PE
PE_NB = len(PE_BINS)
INV_LN2 = 1.4426950408889634
F32 = mybir.dt.float32
BF16 = mybir.dt.bfloat16
N_STAGES = 5

_CACHE = {}


def _pack_params(inputs):
    """[192, 59]: m0(3) m1(9) m2(9) m3(9) m4(3) | b0..b4(13) | f0..f3(12) | med(1)"""
    cols = []
    for i in range(N_STAGES):
        cols.append(np.asarray(inputs[f"matrix{i}"], np.float32).reshape(C, -1))
    for i in range(N_STAGES):
        cols.append(np.asarray(inputs[f"bias{i}"], np.float32).reshape(C, -1))
    for i in range(4):
        cols.append(np.asarray(inputs[f"factor{i}"], np.float32).reshape(C, -1))
    med = np.asarray(inputs["quantiles"], np.float32)[:, :, 1].reshape(C, 1)
    cols.append(med)
    return np.ascontiguousarray(np.concatenate(cols, axis=1))


# param column indices in the packed array
SP0 = 0            # matrix0: 3 cols
SP = [0, 3, 12, 21, 30]   # start col of matrix_i
BIA = [33, 36, 39, 42, 45]
FAC = 46           # factors: 12 cols (within tf tile: col i*3+j)
MED = 58


def _kgrid():
    ks = np.arange(K_LO, K_HI + 1, dtype=np.float32)
    sb = 0.5 - np.arange(ACT_K_LO, ACT_K_LO + ACT_NB + 1, dtype=np.float32)  # sign-fan biases
    g = np.concatenate([ks - 0.5, ks + 0.5, sb])      # [70 + ACT_NB + 1]
    return np.ascontiguousarray(np.broadcast_to(g, (128, g.size)).copy())


def _build_table(nc, pool, kgt, p, sp, tf):
    """Emit the per-channel MLP chain on [128, 70] tiles.  Returns Tt = ln(lh)."""
    W2 = 2 * NBINS

    def tl(nm):
        return pool.tile([128, W2], F32, tag=nm, name=nm)

    v = tl("tbl_v")
    nc.vector.tensor_scalar(v[:], kgt[:, 0:W2], p[:, MED:MED+1], None, ALU.add)
    h = [tl(f"tbl_h{j}") for j in range(3)]
    g = [tl(f"tbl_g{j}") for j in range(3)]
    tn = [tl(f"tbl_tn{j}") for j in range(3)]
    # stage 0: 1 -> 3, then tanh gate
    for j in range(3):
        nc.vector.tensor_scalar(h[j][:], v[:], sp[:, j:j+1],
                                p[:, BIA[0]+j:BIA[0]+j+1], ALU.mult, ALU.add)
        nc.scalar.activation(tn[j][:], h[j][:], ACT.Tanh)
        nc.vector.scalar_tensor_tensor(h[j][:], tn[j][:], tf[:, j:j+1], h[j][:],
                                       ALU.mult, ALU.add)
    # stages 1..3: 3 -> 3 with tanh gate
    for i in range(1, 4):
        for j in range(3):
            c0 = SP[i] + 3 * j
            nc.vector.tensor_scalar(g[j][:], h[0][:], sp[:, c0:c0+1],
                                    p[:, BIA[i]+j:BIA[i]+j+1], ALU.mult, ALU.add)
            nc.vector.scalar_tensor_tensor(g[j][:], h[1][:], sp[:, c0+1:c0+2],
                                           g[j][:], ALU.mult, ALU.add)
            nc.vector.scalar_tensor_tensor(g[j][:], h[2][:], sp[:, c0+2:c0+3],
                                           g[j][:], ALU.mult, ALU.add)
            nc.scalar.activation(tn[j][:], g[j][:], ACT.Tanh)
            nc.vector.scalar_tensor_tensor(g[j][:], tn[j][:], tf[:, 3*i+j:3*i+j+1],
                                           g[j][:], ALU.mult, ALU.add)
        h, g = g, h
    # stage 4: 3 -> 1
    o = tl("tbl_o")
    nc.vector.tensor_scalar(o[:], h[0][:], sp[:, SP[4]:SP[4]+1],
                            p[:, BIA[4]:BIA[4]+1], ALU.mult, ALU.add)
    nc.vector.scalar_tensor_tensor(o[:], h[1][:], sp[:, SP[4]+1:SP[4]+2], o[:],
                                   ALU.mult, ALU.add)
    nc.vector.scalar_tensor_tensor(o[:], h[2][:], sp[:, SP[4]+2:SP[4]+3], o[:],
                                   ALU.mult, ALU.add)
    lo, up = o[:, 0:NBINS], o[:, NBINS:W2]
    t1 = pool.tile([128, NBINS], F32)
    nc.vector.tensor_tensor(t1[:], lo, up, ALU.add)
    sg = pool.tile([128, NBINS], F32)
    nc.scalar.activation(sg[:], t1[:], ACT.Sign, scale=-1.0)   # -sign(lo+up)
    su = pool.tile([128, NBINS], F32)
    sl = pool.tile([128, NBINS], F32)
    nc.vector.tensor_tensor(su[:], up, sg[:], ALU.mult)
    nc.vector.tensor_tensor(sl[:], lo, sg[:], ALU.mult)
    nc.scalar.activation(su[:], su[:], ACT.Sigmoid)
    nc.scalar.activation(sl[:], sl[:], ACT.Sigmoid)
    d = pool.tile([128, NBINS], F32)
    nc.vector.tensor_tensor(d[:], su[:], sl[:], ALU.subtract)
    nc.scalar.activation(d[:], d[:], ACT.Abs)
    nc.vector.tensor_scalar(d[:], d[:], 1e-9, None, ALU.max)
    tt = pool.tile([128, NBINS], F32)
    nc.scalar.activation(tt[:], d[:], ACT.Ln)
    return tt


def build_nc():
    nc = Bass("TRN2", target_bir_lowering=False, debug=False,
              enable_asserts=False, num_devices=N_CORES)
    xs = nc.dram_tensor("xs", [B_PER_CORE, C, NHW], F32, kind="ExternalInput")
    pp = nc.dram_tensor("pp", [C, 59], F32, kind="ExternalInput")
    kg = nc.dram_tensor("kg", [128, 2 * NBINS + ACT_NB + 1], F32, kind="ExternalInput")
    ys = nc.dram_tensor("ys", [B_PER_CORE, C, NHW], F32, kind="ExternalOutput")
    bp = nc.dram_tensor("bp", [128, 1], F32, kind="ExternalOutput")
    bq = nc.dram_tensor("bq", [1, 1], F32, kind="ExternalOutput")

    with ExitStack() as ctx:
        tc = ctx.enter_context(tile.TileContext(nc))
        prm = ctx.enter_context(tc.tile_pool(name="prm", bufs=1))
        tbl = ctx.enter_context(tc.tile_pool(name="tbl", bufs=2))
        xp = ctx.enter_context(tc.tile_pool(name="xp", bufs=2))
        tp = ctx.enter_context(tc.tile_pool(name="tp", bufs=2))
        rp = ctx.enter_context(tc.tile_pool(name="rp", bufs=3))
        scr = ctx.enter_context(tc.tile_pool(name="scr", bufs=1))
        cnt = ctx.enter_context(tc.tile_pool(name="cnt", bufs=1))
        wmp = ctx.enter_context(tc.tile_pool(name="wmp", bufs=4))
        psp = ctx.enter_context(tc.tile_pool(name="psp", bufs=1, space="PSUM"))

        # ---- parameters ----
        pa = prm.tile([128, 59], F32, tag="pa", name="pa")
        nc.sync.dma_start(pa[:], pp.ap()[0:128, :])
        pb = prm.tile([128, 59], F32, tag="pb", name="pb")
        nc.sync.dma_start(pb[0:64, :], pp.ap()[128:192, :])
        nc.sync.dma_start(pb[64:128, :], pp.ap()[128:192, :])
        kgt = prm.tile([128, 2 * NBINS + ACT_NB + 1], F32, tag="kgt", name="kgt")
        nc.sync.dma_start(kgt[:], kg.ap()[:])

        spt, tft, tts, t16 = {}, {}, {}, {}
        omb = {}   # per-layout outp bias col: med - MAGIC
        for L, p in (("A", pa), ("B", pb)):
            ob = prm.tile([128, 1], F32, tag=f"omb{L}", name=f"omb{L}")
            nc.vector.tensor_scalar(ob[:], p[:, MED:MED+1], MAGIC, None, ALU.subtract)
            omb[L] = ob
            s = prm.tile([128, 33], F32, tag=f"sp{L}", name=f"sp{L}")
            # softplus(x) = ln(1 + exp(x)) — no Softplus in the ACT tables here
            nc.scalar.activation(s[:], p[:, 0:33], ACT.Exp)
            nc.vector.tensor_scalar(s[:], s[:], 1.0, None, ALU.add)
            nc.scalar.activation(s[:], s[:], ACT.Ln)
            f = prm.tile([128, 12], F32, tag=f"tf{L}", name=f"tf{L}")
            nc.scalar.activation(f[:], p[:, FAC:FAC+12], ACT.Tanh)
            spt[L], tft[L] = s, f

        # ---- tables (tiny) ----
        for L, p in (("A", pa), ("B", pb)):
            tts[L] = _build_table(nc, tbl, kgt, p, spt[L], tft[L])
            w16 = prm.tile([128, NBINS], mybir.dt.float16, tag=f"t16{L}", name=f"t16{L}")
            nc.vector.tensor_copy(w16[:], tts[L][:])
            t16[L] = w16

        # ---- count accumulators ----
        # ACT sign sums: [128, ACT_NB+1, nchunks]; PE bins accumulate in PSUM
        chunks = [("A", 0), ("A", 1), ("B", None)]
        nch = {"A": 2, "B": 1}
        sa = {L: cnt.tile([128, ACT_NB + 1, nch[L]], F32, tag=f"sa{L}", name=f"sa{L}") for L in "AB"}
        psum = psp.tile([1, NHW], F32, tag="psum", name="psum")
        NSL = NHW // 512

        # ---- main loop ----
        for (L, b) in chunks:
            p = pa if L == "A" else pb
            med = p[:, MED:MED+1]
            x = xp.tile([128, NHW], F32, tag="x", name="x")
            if L == "A":
                nc.sync.dma_start(x[:], xs.ap()[b, 0:128, :])
            else:
                nc.sync.dma_start(x[0:64, :], xs.ap()[0, 128:192, :])
                nc.sync.dma_start(x[64:128, :], xs.ap()[1, 128:192, :])
            t = tp.tile([128, NHW], F32, tag="t", name="t")
            # t = (x - m) + MAGIC  -> integer-valued + MAGIC (exact RNE round)
            nc.vector.tensor_scalar(t[:], x[:], med, MAGIC, ALU.subtract, ALU.add)
            # outputs = t + (m - MAGIC), on ACT to offload DVE
            nc.scalar.activation(x[:], t[:], ACT.Identity, bias=omb[L][:, 0:1])
            if L == "A":
                nc.sync.dma_start(ys.ap()[b, 0:128, :], x[:])
            else:
                nc.sync.dma_start(ys.ap()[0, 128:192, :], x[0:64, :])
                nc.sync.dma_start(ys.ap()[1, 128:192, :], x[64:128, :])
            # r (bf16, clamped) for binning
            r = rp.tile([128, NHW], BF16, tag="r", name="r")
            nc.vector.tensor_scalar(r[:], t[:], MAGIC, None, ALU.subtract)
            rc = rp.tile([128, NHW], BF16, tag="rc", name="rc")
            nc.vector.tensor_scalar(rc[:], r[:], float(K_LO), float(K_HI),
                                    ALU.max, ALU.min)
            ci = b if L == "A" else 0
            # PE bins: T-weighted masks on DVE, summed over channels by PE
            tt = tts[L]
            first_chunk = (L == "A" and b == 0)
            last_chunk = (L == "B")
            w16L = t16[L]
            for i, k in enumerate(PE_BINS):
                kcol = k - K_LO
                wm = wmp.tile([128, NHW], mybir.dt.float16, tag="wm", name="wm")
                nc.vector.tensor_scalar(wm[:], rc[:], float(k), None, ALU.is_equal)
                for s in range(NSL):
                    nc.tensor.matmul(psum[:, s*512:(s+1)*512],
                                     w16L[:, kcol:kcol+1], wm[:, s*512:(s+1)*512],
                                     start=(first_chunk and i == 0),
                                     stop=(last_chunk and i == PE_NB - 1))
            # ACT bins: sign-fan partial sums  S_k = sum sign(r - (k-0.5))
            sact = scr.tile([128, NHW], BF16, tag="sact", name="sact")
            for i in range(ACT_NB + 1):
                k = K_LO + i
                nc.scalar.activation(sact[:], rc[:], ACT.Sign,
                                     bias=kgt[:, 2*NBINS+i:2*NBINS+i+1],
                                     accum_out=sa[L][:, i, ci:ci+1])

        # ---- finalize bits ----
        acc = cnt.tile([128, 1], F32, tag="acc", name="acc")
        first = True
        for L in ("A", "B"):
            sar = cnt.tile([128, ACT_NB + 1], F32, tag=f"sar{L}", name=f"sar{L}")
            if nch[L] > 1:
                nc.vector.tensor_reduce(sar[:], sa[L][:], mybir.AxisListType.X, ALU.add)
            else:
                nc.vector.tensor_copy(sar[:], sa[L][:, :, 0])
            # ACT counts: (S_k - S_{k+1}) / 2
            ca = cnt.tile([128, ACT_NB], F32, tag=f"ca{L}", name=f"ca{L}")
            nc.vector.tensor_tensor(ca[:], sar[:, 0:ACT_NB], sar[:, 1:ACT_NB+1],
                                    ALU.subtract)
            nc.vector.tensor_scalar(ca[:], ca[:], 0.5, None, ALU.mult)
            tt = tts[L]
            a0 = ACT_K_LO - K_LO
            pa2 = cnt.tile([128, ACT_NB], F32, tag=f"pa2{L}", name=f"pa2{L}")
            nc.vector.tensor_tensor(pa2[:], ca[:], tt[:, a0:a0+ACT_NB], ALU.mult)
            rsum2 = cnt.tile([128, 1], F32, tag=f"rs2{L}", name=f"rs2{L}")
            nc.vector.tensor_reduce(rsum2[:], pa2[:], mybir.AxisListType.X, ALU.add)
            if first:
                nc.vector.tensor_copy(acc[:], rsum2[:])
                first = False
            else:
                nc.vector.tensor_tensor(acc[:], acc[:], rsum2[:], ALU.add)
        nc.vector.tensor_scalar(acc[:], acc[:], -INV_LN2, None, ALU.mult)
        nc.sync.dma_start(bp.ap()[:], acc[:])
        # PE partial: per-slice PSUM reduces (overlap with trailing matmuls)
        pesl = cnt.tile([1, NSL], F32, tag="pesl", name="pesl")
        for s in range(NSL):
            nc.vector.tensor_reduce(pesl[:, s:s+1], psum[:, s*512:(s+1)*512],
                                    mybir.AxisListType.X, ALU.add)
        pes = cnt.tile([1, 1], F32, tag="pes", name="pes")
        nc.vector.tensor_reduce(pes[:], pesl[:], mybir.AxisListType.X, ALU.add)
        nc.vector.tensor_scalar(pes[:], pes[:], -INV_LN2, None, ALU.mult)
        nc.sync.dma_start(bq.ap()[:], pes[:])

    _split_multiwaits(nc)
    return nc


def _split_multiwaits(nc):
    """This walrus accepts at most 1 sync wait per instruction (2 for
    EventSemaphore); hoist extras onto preceding same-engine NoOps."""
    for f in nc.m.functions:
        for blk in f.blocks:
            il = blk.instructions
            i = 0
            while i < len(il):
                ins = il[i]
                si = ins.sync_info
                k = 2 if isinstance(ins, mybir.InstEventSemaphore) else 1
                if si is not None and len(si.on_wait) > k:
                    waits = list(si.on_wait)
                    extra, kept = waits[:-k], waits[-k:]
                    for w in extra:
                        nop = mybir.InstNoOp(
                            name=nc.get_next_instruction_name(), ins=[], outs=[])
                        nop.engine = ins.engine
                        nop.sync_info = mybir.SyncInfo(on_wait=[w], on_update=[])
                        il.insert(i, nop)
                        i += 1
                    si.on_wait = kept
                    ins.sync_info = si
                i += 1


def kernel(**inputs):
    if "nc" not in _CACHE:
        _CACHE["nc"] = build_nc()
    nc = _CACHE["nc"]

    x = np.ascontiguousarray(np.asarray(inputs["x"], np.float32))
    pp = _pack_params(inputs)
    kg = _CACHE.setdefault("kg", _kgrid())
    xr = x.reshape(B, C, NHW)
    in_maps = [{"xs": xr[i * B_PER_CORE:(i + 1) * B_PER_CORE],
                "pp": pp, "kg": kg} for i in range(N_CORES)]
    res = bass_utils.run_bass_kernel_spmd(nc, in_maps, core_ids=list(range(N_CORES)))
    out = np.empty((B, C, H, W), np.float32)
    bits = 0.0
    for i in range(N_CORES):
        out[i * B_PER_CORE:(i + 1) * B_PER_CORE] = \
            res.results[i]["ys"].reshape(B_PER_CORE, C, H, W)
        bits += float(res.results[i]["bp"].sum(dtype=np.float64))
        bits += float(res.results[i]["bq"].sum(dtype=np.float64))
    return out, np.float32(bits)
